# revision 11
# baseline (speedup 1.0000x reference)
"""BertAttention (quantized, eval) Trainium2 kernel.

Data-parallel over batch: 8 batch elements -> 8 NeuronCores, one full
attention layer per core, no collectives.  Returns the same tuple as the
reference: (attention_output, scores, probs).

Key ideas:
- Every fake-quantized tensor (weights, q, k, v, probs, ctx) is
  round(x/s) * s with round(x/s) an integer in [-127, 127].  We carry the
  integers (exact in bf16) through the TensorEngine, so QK^T, PV and the
  output projection are exact integer matmuls accumulated in f32 PSUM
  (max |sum| < 2^24); the f32 scales are applied in the epilogues.
- round-half-to-even == (x + 1.5*2^23) - 1.5*2^23 in f32, one 2-op
  tensor_scalar instruction (matches jnp.round for |x| <= 2^22).
- scores are computed in both [q,k] layout (scores/probs outputs, softmax
  row sums via the ACT engine's accum_out) and [k,q] layout (so the PV
  matmul gets probs^T without transposing the 64MB probs tensor).  Both
  layouts reuse the same Q^T / K^T integer operands.
"""

import sys

sys.path.insert(0, "/opt/trn_rl_repo")

import numpy as np

import concourse.bass as bass
import concourse.bass_isa as bass_isa
import concourse.tile as tile
from concourse import bacc, mybir
from concourse.bass_utils import run_bass_kernel_spmd
from concourse.masks import make_identity

H = 1024
S = 1024
NH = 16
D = 64
P = 128
OC = H // P  # 8 chunks of 128 output channels
TC = S // P  # 8 chunks of 128 tokens
F32 = mybir.dt.float32
F32R = mybir.dt.float32r
BF16 = mybir.dt.bfloat16

CLIP = np.float32(2.5)
N_LEVELS = np.float32(127.0)
S_ACT = np.float32(CLIP / N_LEVELS)  # activation quant step
INV_S_ACT = float(np.float32(1.0) / S_ACT)  # ~50.8
C_QK = float(np.float32(S_ACT * S_ACT) / np.float32(8.0))  # sqrt(D) == 8
C_MAGIC = 12582912.0  # 1.5 * 2**23: f32 round-to-nearest-even bias
LN_EPS = 1e-12

ADD = mybir.AluOpType.add
SUB = mybir.AluOpType.subtract
MULT = mybir.AluOpType.mult
MIN = mybir.AluOpType.min
MAX = mybir.AluOpType.max
BYPASS = mybir.AluOpType.bypass
Exp = mybir.ActivationFunctionType.Exp
Copy = mybir.ActivationFunctionType.Copy
Sqrt = mybir.ActivationFunctionType.Sqrt
XY_AXIS = mybir.AxisListType.XY


def _transpose_to(nc, psum_pool, dst, src, ident):
    """PE-transpose [P, C, J*P] -> [P, J, C*P]: each 128x128 tile
    src[:, c, j*P:+P] lands transposed in dst[:, j, c*P:+P]."""
    n_c = src.shape[1]
    n_j = src.shape[2] // P
    for c in range(n_c):
        for j in range(n_j):
            pt = psum_pool.tile([P, P], F32, tag="tr_psum")
            nc.tensor.transpose(pt[:], src[:, c, j * P : (j + 1) * P], ident)
            nc.any.tensor_copy(out=dst[:, j, c * P : (c + 1) * P], in_=pt[:])


def build(mask_nonzero: bool):
    nc = bacc.Bacc("TRN2", target_bir_lowering=False, debug=False, num_devices=8)

    hid = nc.dram_tensor("hidden", [S, H], F32, kind="ExternalInput").ap()
    msk = nc.dram_tensor("mask", [S], F32, kind="ExternalInput").ap()
    w_dr = {}
    b_dr = {}
    for nm, w_name, b_name in (
        ("q", "Wq", "bq"),
        ("k", "Wk", "bk"),
        ("v", "Wv", "bv"),
        ("o", "Wo", "bo"),
    ):
        w_dr[nm] = nc.dram_tensor(w_name, [H, H], F32, kind="ExternalInput").ap()
        b_dr[nm] = nc.dram_tensor(b_name, [H], F32, kind="ExternalInput").ap()
    g_ln = nc.dram_tensor("ln_gamma", [H], F32, kind="ExternalInput").ap()
    beta_ln = nc.dram_tensor("ln_beta", [H], F32, kind="ExternalInput").ap()

    out_attn = nc.dram_tensor("attn_out", [S, H], F32, kind="ExternalOutput").ap()
    out_scores = nc.dram_tensor("scores", [NH, S, S], F32, kind="ExternalOutput").ap()
    out_probs = nc.dram_tensor("probs", [NH, S, S], F32, kind="ExternalOutput").ap()

    with tile.TileContext(nc) as tc:
        with tc.tile_pool(name="pers", bufs=1) as pers:
            # -------- persistent tensors (span several phases) --------
            qT = pers.tile([P, OC, S], BF16, tag="qT")  # q^T ints [o, t]
            kT = pers.tile([P, OC, S], BF16, tag="kT")
            v_int = pers.tile([P, TC, H], BF16, tag="v_int")  # v ints [t, o]
            a2 = pers.tile([P, OC, S], BF16, tag="a2")  # ctx_q^T ints [H, t]
            woT = pers.tile([P, OC, H], BF16, tag="woT")  # Wo'^T ints [i, o]
            ident = pers.tile([P, P], F32, tag="ident")
            make_identity(nc, ident[:])
            # per-o-channel bias carriers ([o%128, oc]) pre-scaled by 1/s
            b50 = {}
            for nm in ("q", "k"):
                braw = pers.tile([P, OC], F32, tag=f"braw_{nm}")
                nc.sync.dma_start(braw[:], b_dr[nm].rearrange("(oc p) -> p oc", p=P))
                b50[nm] = pers.tile([P, OC], F32, tag=f"b50_{nm}", name=f"b50_{nm}")
                nc.vector.tensor_scalar(b50[nm][:], braw[:], INV_S_ACT, None, MULT)
            bv_row = pers.tile([1, H], F32, tag="bv_row")
            nc.sync.dma_start(bv_row[:], b_dr["v"][None, :])
            bv50 = pers.tile([1, H], F32, tag="bv50")
            nc.vector.tensor_scalar(bv50[:], bv_row[:], INV_S_ACT, None, MULT)
            bv50_rep = pers.tile([P, H], F32, tag="bv50_rep")
            nc.gpsimd.partition_broadcast(bv50_rep[:], bv50[0:1, :])
            bo_row = pers.tile([1, H], F32, tag="bo_row")
            nc.sync.dma_start(bo_row[:], b_dr["o"][None, :])
            g_row = pers.tile([1, H], F32, tag="g_row")
            nc.sync.dma_start(g_row[:], g_ln[None, :])
            beta_row = pers.tile([1, H], F32, tag="beta_row")
            nc.sync.dma_start(beta_row[:], beta_ln[None, :])
            if mask_nonzero:
                mask_pp = pers.tile([P, TC], F32, tag="mask_pp")  # [k%128, kc]
                nc.sync.dma_start(mask_pp[:], msk.rearrange("(kc p) -> p kc", p=P))
                mask_row = pers.tile([1, S], F32, tag="mask_row")
                nc.sync.dma_start(mask_row[:], msk[None, :])
                maskc = pers.tile([1, S], F32, tag="maskc")  # mask / C_QK
                nc.vector.tensor_scalar(maskc[:], mask_row[:], 1.0 / C_QK, None, MULT)
                ones_row = pers.tile([1, S], F32, tag="ones_row")
                nc.vector.memset(ones_row[:], 1.0)
            # weight scales (s_w = absmax/127), one [P,1] carrier per weight
            w_scale = {
                nm: pers.tile([P, 1], F32, tag=f"ws_{nm}", name=f"ws_{nm}")
                for nm in w_dr
            }

            # -------- phase A/B: x^T, quantized weights, projections --
            with (
                tc.tile_pool(name="xT_pool", bufs=1) as xTp,
                tc.tile_pool(name="tr_psum", bufs=4, space="PSUM") as trp,
                tc.tile_pool(name="mm_psum", bufs=4, space="PSUM") as mmp,
                tc.tile_pool(name="epi", bufs=2) as epi,
                tc.tile_pool(name="wstage", bufs=1) as wsp,
                tc.tile_pool(name="wT_pool", bufs=1) as wTp,
                tc.tile_pool(name="stats", bufs=1) as stats,
            ):
                xT = xTp.tile([P, OC, S], F32R, tag="xT")  # x^T [i, t]
                x_sb = wsp.tile([P, TC, H], F32, tag="w_sb", name="x_sb")
                nc.sync.dma_start(x_sb[:], hid.rearrange("(t p) i -> p t i", p=P))
                _transpose_to(nc, trp, xT, x_sb, ident[:])

                for nm in ("o", "v", "q", "k"):
                    # ---- load + per-tensor quantize (in place) ----
                    w_sb = wsp.tile([P, OC, H], F32, tag="w_sb")
                    nc.sync.dma_start(
                        w_sb[:], w_dr[nm].rearrange("(oc p) i -> p oc i", p=P)
                    )
                    amax_p = stats.tile([P, 1], F32, tag=f"amax_p_{nm}")
                    nc.vector.tensor_reduce(
                        amax_p[:], w_sb[:], XY_AXIS, MAX, apply_absolute_value=True
                    )
                    amax = stats.tile([P, 1], F32, tag=f"amax_{nm}")
                    nc.gpsimd.partition_all_reduce(
                        amax[:], amax_p[:], channels=P, reduce_op=bass_isa.ReduceOp.max
                    )
                    nc.vector.tensor_scalar(
                        w_scale[nm][:], amax[:], 1.0 / 127.0, None, MULT
                    )
                    inv_s = stats.tile([P, 1], F32, tag=f"invs_{nm}")
                    nc.vector.reciprocal(inv_s[:], amax[:])
                    nc.vector.tensor_scalar(inv_s[:], inv_s[:], 127.0, None, MULT)
                    nc.vector.tensor_scalar(
                        w_sb[:], w_sb[:], inv_s[:], C_MAGIC, MULT, ADD
                    )
                    nc.vector.tensor_scalar(w_sb[:], w_sb[:], C_MAGIC, None, SUB)
                    # ---- transpose W' ints -> [i, o] ----
                    wT = wTp.tile([P, OC, H], F32R, tag="wT")
                    _transpose_to(nc, trp, wT, w_sb, ident[:])
                    if nm == "o":
                        nc.any.tensor_copy(out=woT[:], in_=wT[:])
                        continue
                    # swi = s_w / s_act  (fold weight scale + act quant step)
                    swi = stats.tile([P, 1], F32, tag=f"swi_{nm}")
                    nc.vector.tensor_scalar(
                        swi[:], w_scale[nm][:], INV_S_ACT, None, MULT
                    )
                    if nm in ("q", "k"):
                        out_int = qT if nm == "q" else kT
                        # GEMM -> [o_p, t_f]
                        for oc in range(OC):
                            for th in range(2):
                                ps = mmp.tile([P, 512], F32, tag="proj_ps")
                                for ic in range(OC):
                                    nc.tensor.matmul(
                                        ps[:],
                                        wT[:, ic, oc * P : (oc + 1) * P],
                                        xT[:, ic, th * 512 : (th + 1) * 512],
                                        start=(ic == 0),
                                        stop=(ic == OC - 1),
                                    )
                                t1 = epi.tile([P, 512], F32, tag="prj_t1")
                                nc.vector.tensor_scalar(
                                    t1[:],
                                    ps[:],
                                    swi[:],
                                    b50[nm][:, oc : oc + 1],
                                    MULT,
                                    ADD,
                                )
                                nc.gpsimd.tensor_scalar(
                                    t1[:], t1[:], 127.0, -127.0, MIN, MAX
                                )
                                nc.gpsimd.tensor_scalar(
                                    out_int[:, oc, th * 512 : (th + 1) * 512],
                                    t1[:],
                                    C_MAGIC,
                                    C_MAGIC,
                                    ADD,
                                    SUB,
                                )
                    else:  # v: GEMM -> [t_p, o_f]
                        for tcc in range(TC):
                            for oh in range(2):
                                ps = mmp.tile([P, 512], F32, tag="proj_ps")
                                for ic in range(OC):
                                    nc.tensor.matmul(
                                        ps[:],
                                        xT[:, ic, tcc * P : (tcc + 1) * P],
                                        wT[:, ic, oh * 512 : (oh + 1) * 512],
                                        start=(ic == 0),
                                        stop=(ic == OC - 1),
                                    )
                                t1 = epi.tile([P, 512], F32, tag="prj_t1")
                                nc.vector.scalar_tensor_tensor(
                                    t1[:],
                                    ps[:],
                                    swi[:],
                                    bv50_rep[:, oh * 512 : (oh + 1) * 512],
                                    MULT,
                                    ADD,
                                )
                                nc.gpsimd.tensor_scalar(
                                    t1[:], t1[:], 127.0, -127.0, MIN, MAX
                                )
                                nc.gpsimd.tensor_scalar(
                                    v_int[:, tcc, oh * 512 : (oh + 1) * 512],
                                    t1[:],
                                    C_MAGIC,
                                    C_MAGIC,
                                    ADD,
                                    SUB,
                                )

            # -------- per-head attention ------------------------------
            with (
                tc.tile_pool(name="rr_dram", bufs=2, space="DRAM") as rrdp,
                tc.tile_pool(name="l1_psum", bufs=2, space="PSUM") as p1pool,
                tc.tile_pool(name="l2_psum", bufs=1, space="PSUM") as p2pool,
                tc.tile_pool(name="ctx_psum", bufs=1, space="PSUM") as pcpool,
                tc.tile_pool(name="l1", bufs=3) as l1pool,
                tc.tile_pool(name="l2", bufs=3) as l2pool,
                tc.tile_pool(name="hstat", bufs=3) as hstat,
                tc.tile_pool(name="ctxepi", bufs=2) as ctxepi,
            ):
                ctx_ps = None
                for h in range(NH):
                    hp = h % 2
                    oc_h = h // 2
                    plo = D * hp
                    qT_h = qT[plo : plo + D, oc_h, :]  # [64, 1024] ints
                    kT_h = kT[plo : plo + D, oc_h, :]

                    rs = hstat.tile([P, TC], F32, tag="rs")
                    rr = hstat.tile([P, TC], F32, tag="rr")
                    rr50 = hstat.tile([P, TC], F32, tag="rr50")
                    rrT = hstat.tile([1, S], F32, tag="rrT")

                    # ---- layout 1: scores [q, k] ----
                    for qc in range(TC):
                        ps1 = p1pool.tile([P, S], F32, tag="ps1")
                        for kh in range(2):
                            nc.tensor.matmul(
                                ps1[:, kh * 512 : (kh + 1) * 512],
                                qT_h[:, qc * P : (qc + 1) * P],
                                kT_h[:, kh * 512 : (kh + 1) * 512],
                                start=True,
                                stop=not mask_nonzero,
                            )
                            if mask_nonzero:
                                nc.tensor.matmul(
                                    ps1[:, kh * 512 : (kh + 1) * 512],
                                    ones_row[0:1, qc * P : (qc + 1) * P],
                                    maskc[0:1, kh * 512 : (kh + 1) * 512],
                                    start=False,
                                    stop=True,
                                    skip_group_check=True,
                                )
                        sc_t = l1pool.tile([P, S], F32, tag="sc")
                        nc.scalar.activation(sc_t[:], ps1[:], Copy, scale=C_QK)
                        nc.sync.dma_start(
                            out_scores[h, qc * P : (qc + 1) * P, :], sc_t[:]
                        )
                        un_t = l1pool.tile([P, S], F32, tag="un")
                        nc.scalar.activation(
                            un_t[:],
                            ps1[:],
                            Exp,
                            scale=C_QK,
                            accum_out=rs[:, qc : qc + 1],
                        )
                        nc.vector.reciprocal(rr[:, qc : qc + 1], rs[:, qc : qc + 1])
                        pr_t = l1pool.tile([P, S], F32, tag="pr")
                        nc.vector.tensor_scalar(
                            pr_t[:], un_t[:], rr[:, qc : qc + 1], None, MULT
                        )
                        nc.sync.dma_start(
                            out_probs[h, qc * P : (qc + 1) * P, :], pr_t[:]
                        )

                    nc.vector.tensor_scalar(rr50[:], rr[:], INV_S_ACT, None, MULT)
                    # scatter [128, TC] -> flat [S] (t = qc*128 + p) via DRAM
                    rr_d = rrdp.tile([S], F32, tag="rr_d")
                    with nc.allow_non_contiguous_dma(reason="tiny per-head stats"):
                        nc.sync.dma_start(
                            rr_d[:].rearrange("(c p) -> p c", p=P), rr50[:, :]
                        )
                    nc.sync.dma_start(rrT[:], rr_d[None, :])
                    rrT_rep = hstat.tile([P, S], F32, tag="rrT_rep")
                    nc.gpsimd.partition_broadcast(rrT_rep[:], rrT[0:1, :])

                    # ---- layout 2: scores^T [k, q] feeding PV ----
                    if hp == 0:
                        ctx_ps = pcpool.tile([P, S], F32, tag="ctx_ps")
                    for kc in range(TC):
                        ps2 = p2pool.tile([P, S], F32, tag="ps2")
                        for qh in range(2):
                            nc.tensor.matmul(
                                ps2[:, qh * 512 : (qh + 1) * 512],
                                kT_h[:, kc * P : (kc + 1) * P],
                                qT_h[:, qh * 512 : (qh + 1) * 512],
                                start=True,
                                stop=True,
                            )
                        un2_t = l2pool.tile([P, S], F32, tag="un2")
                        if mask_nonzero:
                            nc.scalar.activation(
                                un2_t[:],
                                ps2[:],
                                Exp,
                                scale=C_QK,
                                bias=mask_pp[:, kc : kc + 1],
                            )
                        else:
                            nc.scalar.activation(un2_t[:], ps2[:], Exp, scale=C_QK)
                        t2_t = l2pool.tile([P, S], F32, tag="t2")
                        nc.vector.tensor_tensor(t2_t[:], un2_t[:], rrT_rep[:], MULT)
                        aT_t = l2pool.tile([P, S], BF16, tag="aT")
                        nc.gpsimd.tensor_scalar(
                            aT_t[:], t2_t[:], C_MAGIC, C_MAGIC, ADD, SUB
                        )
                        for qh in range(2):
                            nc.tensor.matmul(
                                ctx_ps[plo : plo + D, qh * 512 : (qh + 1) * 512],
                                v_int[:, kc, h * D : (h + 1) * D],
                                aT_t[:, qh * 512 : (qh + 1) * 512],
                                start=(kc == 0),
                                stop=(kc == TC - 1),
                                skip_group_check=True,
                            )

                    if hp == 1:
                        # fake-quant ctx^T pair -> a2[:, oc_h, :] (ints, bf16)
                        c_pv = float(np.float32(S_ACT * S_ACT) * np.float32(INV_S_ACT))
                        e1 = ctxepi.tile([P, S], F32, tag="ce1")
                        nc.vector.tensor_scalar(
                            e1[:], ctx_ps[:], c_pv, 127.0, MULT, MIN
                        )
                        nc.gpsimd.tensor_scalar(
                            e1[:], e1[:], -127.0, C_MAGIC, MAX, ADD
                        )
                        nc.gpsimd.tensor_scalar(
                            a2[:, oc_h, :], e1[:], C_MAGIC, None, SUB
                        )

            # -------- output projection + residual + LayerNorm --------
            with (
                tc.tile_pool(name="o_psum", bufs=2, space="PSUM") as opool,
                tc.tile_pool(name="fin", bufs=3) as fin,
                tc.tile_pool(name="rows", bufs=1) as rows,
                tc.tile_pool(name="lnstat", bufs=2) as lnstat,
            ):
                bo_rep = rows.tile([P, H], F32, tag="bo_rep")
                nc.gpsimd.partition_broadcast(bo_rep[:], bo_row[0:1, :])
                g_rep = rows.tile([P, H], F32, tag="g_rep")
                nc.gpsimd.partition_broadcast(g_rep[:], g_row[0:1, :])
                beta_rep = rows.tile([P, H], F32, tag="beta_rep")
                nc.gpsimd.partition_broadcast(beta_rep[:], beta_row[0:1, :])
                c_o = lnstat.tile([P, 1], F32, tag="c_o")  # s_ctx * s_wo
                nc.vector.tensor_scalar(
                    c_o[:], w_scale["o"][:], float(S_ACT), None, MULT
                )
                eps_t = lnstat.tile([P, 1], F32, tag="eps_t")
                nc.vector.memset(eps_t[:], LN_EPS)
                for tcc in range(TC):
                    ps = opool.tile([P, H], F32, tag="ops")
                    for oh in range(2):
                        for hc in range(OC):
                            nc.tensor.matmul(
                                ps[:, oh * 512 : (oh + 1) * 512],
                                a2[:, hc, tcc * P : (tcc + 1) * P],
                                woT[:, hc, oh * 512 : (oh + 1) * 512],
                                start=(hc == 0),
                                stop=(hc == OC - 1),
                            )
                    hid_t = fin.tile([P, H], F32, tag="hid_t")
                    nc.sync.dma_start(hid_t[:], hid[tcc * P : (tcc + 1) * P, :])
                    nc.vector.tensor_tensor(hid_t[:], hid_t[:], bo_rep[:], ADD)
                    x_t = fin.tile([P, H], F32, tag="x_t")
                    nc.vector.scalar_tensor_tensor(
                        x_t[:], ps[:], c_o[:], hid_t[:], MULT, ADD
                    )
                    # LayerNorm over the free (H) dim
                    xsum = lnstat.tile([P, 1], F32, tag="xsum")
                    nc.vector.tensor_reduce(xsum[:], x_t[:], XY_AXIS, ADD)
                    mu = lnstat.tile([P, 1], F32, tag="mu")
                    nc.vector.tensor_scalar(mu[:], xsum[:], 1.0 / H, None, MULT)
                    xc_t = fin.tile([P, H], F32, tag="xc_t")
                    varsum = lnstat.tile([P, 1], F32, tag="varsum")
                    nc.vector.tensor_scalar(xc_t[:], x_t[:], mu[:], None, SUB)
                    # square into the spent hid_t slot; only accum_out matters
                    nc.vector.scalar_tensor_tensor(
                        hid_t[:], xc_t[:], 1.0, xc_t[:], MULT, MULT,
                        accum_out=varsum[:],
                    )
                    std = lnstat.tile([P, 1], F32, tag="std")
                    nc.scalar.activation(
                        std[:], varsum[:], Sqrt, scale=1.0 / H, bias=eps_t[:]
                    )
                    rstd = lnstat.tile([P, 1], F32, tag="rstd")
                    nc.vector.reciprocal(rstd[:], std[:])
                    nc.vector.tensor_scalar(xc_t[:], xc_t[:], rstd[:], None, MULT)
                    nc.vector.tensor_tensor(xc_t[:], xc_t[:], g_rep[:], MULT)
                    nc.vector.tensor_tensor(xc_t[:], xc_t[:], beta_rep[:], ADD)
                    nc.sync.dma_start(out_attn[tcc * P : (tcc + 1) * P, :], xc_t[:])

    nc.compile()
    return nc


_CACHE = {}


def _get_nc(mask_nonzero):
    key = bool(mask_nonzero)
    if key not in _CACHE:
        _CACHE[key] = build(key)
    return _CACHE[key]


def kernel(
    hidden_states,
    attention_mask,
    Wq,
    bq,
    Wk,
    bk,
    Wv,
    bv,
    Wo,
    bo,
    ln_gamma,
    ln_beta,
    trace=False,
    **trace_kwargs,
):
    hidden_states = np.asarray(hidden_states, dtype=np.float32)
    attention_mask = np.asarray(attention_mask, dtype=np.float32)
    B = hidden_states.shape[0]
    assert B == 8 and hidden_states.shape[1:] == (S, H)

    mask_nonzero = bool(np.any(attention_mask != 0.0))
    nc = _get_nc(mask_nonzero)

    shared = {
        "Wq": np.ascontiguousarray(Wq, np.float32),
        "Wk": np.ascontiguousarray(Wk, np.float32),
        "Wv": np.ascontiguousarray(Wv, np.float32),
        "Wo": np.ascontiguousarray(Wo, np.float32),
        "bq": np.ascontiguousarray(bq, np.float32),
        "bk": np.ascontiguousarray(bk, np.float32),
        "bv": np.ascontiguousarray(bv, np.float32),
        "bo": np.ascontiguousarray(bo, np.float32),
        "ln_gamma": np.ascontiguousarray(ln_gamma, np.float32),
        "ln_beta": np.ascontiguousarray(ln_beta, np.float32),
    }
    in_maps = []
    for b in range(B):
        m = dict(shared)
        m["hidden"] = np.ascontiguousarray(hidden_states[b])
        m["mask"] = np.ascontiguousarray(
            np.broadcast_to(attention_mask[b], (1, 1, S)).reshape(S)
        )
        in_maps.append(m)

    res = run_bass_kernel_spmd(
        nc, in_maps, core_ids=list(range(8)), trace=trace, **trace_kwargs
    )
    attn = np.stack([res.results[b]["attn_out"] for b in range(B)])
    scores = np.stack([res.results[b]["scores"] for b in range(B)])
    probs = np.stack([res.results[b]["probs"] for b in range(B)])
    kernel.last_results = res
    return attn, scores, probs


# revision 12
# speedup vs baseline: 2.4628x; 2.4628x over previous
"""BertAttention (quantized, eval) Trainium2 kernel.

Data-parallel over batch: 8 batch elements -> 8 NeuronCores, one full
attention layer per core, no collectives.  Returns the same tuple as the
reference: (attention_output, scores, probs).

Key ideas:
- Every fake-quantized tensor (weights, q, k, v, probs, ctx) is
  round(x/s) * s with round(x/s) an integer in [-127, 127].  We carry the
  integers (exact in bf16) through the TensorEngine, so QK^T, PV and the
  output projection are exact integer matmuls accumulated in f32 PSUM
  (max |sum| < 2^24); the f32 scales are applied in the epilogues.
- round-half-to-even == (x + 1.5*2^23) - 1.5*2^23 in f32, one 2-op
  tensor_scalar instruction (matches jnp.round for |x| <= 2^22).
- scores are computed in both [q,k] layout (scores/probs outputs, softmax
  row sums via the ACT engine's accum_out) and [k,q] layout (so the PV
  matmul gets probs^T without transposing the 64MB probs tensor).  Both
  layouts reuse the same Q^T / K^T integer operands.
"""

import sys

sys.path.insert(0, "/opt/trn_rl_repo")

import numpy as np

import concourse.bass as bass
import concourse.bass_isa as bass_isa
import concourse.tile as tile
from concourse import bacc, mybir
from concourse.bass_utils import run_bass_kernel_spmd
from concourse.masks import make_identity

H = 1024
S = 1024
NH = 16
D = 64
P = 128
OC = H // P  # 8 chunks of 128 output channels
TC = S // P  # 8 chunks of 128 tokens
F32 = mybir.dt.float32
F32R = mybir.dt.float32r
BF16 = mybir.dt.bfloat16

CLIP = np.float32(2.5)
N_LEVELS = np.float32(127.0)
S_ACT = np.float32(CLIP / N_LEVELS)  # activation quant step
INV_S_ACT = float(np.float32(1.0) / S_ACT)  # ~50.8
C_QK = float(np.float32(S_ACT * S_ACT) / np.float32(8.0))  # sqrt(D) == 8
C_MAGIC = 12582912.0  # 1.5 * 2**23: f32 round-to-nearest-even bias
LN_EPS = 1e-12

ADD = mybir.AluOpType.add
SUB = mybir.AluOpType.subtract
MULT = mybir.AluOpType.mult
MIN = mybir.AluOpType.min
MAX = mybir.AluOpType.max
BYPASS = mybir.AluOpType.bypass
Exp = mybir.ActivationFunctionType.Exp
Copy = mybir.ActivationFunctionType.Copy
Sqrt = mybir.ActivationFunctionType.Sqrt
XY_AXIS = mybir.AxisListType.XY


def _transpose_to(nc, psum_pool, dst, src, ident):
    """PE-transpose [P, C, J*P] -> [P, J, C*P]: each 128x128 tile
    src[:, c, j*P:+P] lands transposed in dst[:, j, c*P:+P]."""
    n_c = src.shape[1]
    n_j = src.shape[2] // P
    for c in range(n_c):
        for j in range(n_j):
            pt = psum_pool.tile([P, P], F32, tag="tr_psum")
            nc.tensor.transpose(pt[:], src[:, c, j * P : (j + 1) * P], ident)
            nc.any.tensor_copy(out=dst[:, j, c * P : (c + 1) * P], in_=pt[:])


def build(mask_nonzero: bool):
    nc = bacc.Bacc("TRN2", target_bir_lowering=False, debug=False, num_devices=8)

    hid = nc.dram_tensor("hidden", [S, H], F32, kind="ExternalInput").ap()
    msk = nc.dram_tensor("mask", [S], F32, kind="ExternalInput").ap()
    w_dr = {}
    b_dr = {}
    for nm, w_name, b_name in (
        ("q", "Wq", "bq"),
        ("k", "Wk", "bk"),
        ("v", "Wv", "bv"),
        ("o", "Wo", "bo"),
    ):
        w_dr[nm] = nc.dram_tensor(w_name, [H, H], F32, kind="ExternalInput").ap()
        b_dr[nm] = nc.dram_tensor(b_name, [H], F32, kind="ExternalInput").ap()
    g_ln = nc.dram_tensor("ln_gamma", [H], F32, kind="ExternalInput").ap()
    beta_ln = nc.dram_tensor("ln_beta", [H], F32, kind="ExternalInput").ap()

    out_attn = nc.dram_tensor("attn_out", [S, H], F32, kind="ExternalOutput").ap()
    out_scores = nc.dram_tensor("scores", [NH, S, S], F32, kind="ExternalOutput").ap()
    out_probs = nc.dram_tensor("probs", [NH, S, S], F32, kind="ExternalOutput").ap()

    with tile.TileContext(nc) as tc:
        with tc.tile_pool(name="pers", bufs=1) as pers:
            # -------- persistent tensors (span several phases) --------
            qT = pers.tile([P, OC, S], BF16, tag="qT")  # q^T ints [o, t]
            kT = pers.tile([P, OC, S], BF16, tag="kT")
            v_int = pers.tile([P, TC, H], BF16, tag="v_int")  # v ints [t, o]
            a2 = pers.tile([P, OC, S], BF16, tag="a2")  # ctx_q^T ints [H, t]
            woT = pers.tile([P, OC, H], BF16, tag="woT")  # Wo'^T ints [i, o]
            ident = pers.tile([P, P], F32, tag="ident")
            make_identity(nc, ident[:])
            # per-o-channel bias carriers ([o%128, oc]) pre-scaled by 1/s
            b50 = {}
            for nm in ("q", "k"):
                braw = pers.tile([P, OC], F32, tag=f"braw_{nm}")
                nc.sync.dma_start(braw[:], b_dr[nm].rearrange("(oc p) -> p oc", p=P))
                b50[nm] = pers.tile([P, OC], F32, tag=f"b50_{nm}", name=f"b50_{nm}")
                nc.vector.tensor_scalar(b50[nm][:], braw[:], INV_S_ACT, None, MULT)
            bv_row = pers.tile([1, H], F32, tag="bv_row")
            nc.sync.dma_start(bv_row[:], b_dr["v"][None, :])
            bv50 = pers.tile([1, H], F32, tag="bv50")
            nc.vector.tensor_scalar(bv50[:], bv_row[:], INV_S_ACT, None, MULT)
            bv50_rep = pers.tile([P, H], F32, tag="bv50_rep")
            nc.gpsimd.partition_broadcast(bv50_rep[:], bv50[0:1, :])
            bo_row = pers.tile([1, H], F32, tag="bo_row")
            nc.sync.dma_start(bo_row[:], b_dr["o"][None, :])
            g_row = pers.tile([1, H], F32, tag="g_row")
            nc.sync.dma_start(g_row[:], g_ln[None, :])
            beta_row = pers.tile([1, H], F32, tag="beta_row")
            nc.sync.dma_start(beta_row[:], beta_ln[None, :])
            if mask_nonzero:
                mask_pp = pers.tile([P, TC], F32, tag="mask_pp")  # [k%128, kc]
                nc.sync.dma_start(mask_pp[:], msk.rearrange("(kc p) -> p kc", p=P))
                mask_row = pers.tile([1, S], F32, tag="mask_row")
                nc.sync.dma_start(mask_row[:], msk[None, :])
                maskc = pers.tile([1, S], F32, tag="maskc")  # mask / C_QK
                nc.vector.tensor_scalar(maskc[:], mask_row[:], 1.0 / C_QK, None, MULT)
                ones_row = pers.tile([1, S], F32, tag="ones_row")
                nc.vector.memset(ones_row[:], 1.0)
            # weight scales (s_w = absmax/127), one [P,1] carrier per weight
            w_scale = {
                nm: pers.tile([P, 1], F32, tag=f"ws_{nm}", name=f"ws_{nm}")
                for nm in w_dr
            }

            # -------- phase A/B: x^T, quantized weights, projections --
            with (
                tc.tile_pool(name="xT_pool", bufs=1) as xTp,
                tc.tile_pool(name="tr_psum", bufs=4, space="PSUM") as trp,
                tc.tile_pool(name="mm_psum", bufs=4, space="PSUM") as mmp,
                tc.tile_pool(name="epi", bufs=2) as epi,
                tc.tile_pool(name="wstage", bufs=1) as wsp,
                tc.tile_pool(name="wT_pool", bufs=1) as wTp,
                tc.tile_pool(name="stats", bufs=1) as stats,
            ):
                xT = xTp.tile([P, OC, S], F32R, tag="xT")  # x^T [i, t]
                x_sb = wsp.tile([P, TC, H], F32, tag="w_sb", name="x_sb")
                nc.sync.dma_start(x_sb[:], hid.rearrange("(t p) i -> p t i", p=P))
                _transpose_to(nc, trp, xT, x_sb, ident[:])

                for nm in ("o", "v", "q", "k"):
                    # ---- load + per-tensor quantize (in place) ----
                    w_sb = wsp.tile([P, OC, H], F32, tag="w_sb")
                    nc.sync.dma_start(
                        w_sb[:], w_dr[nm].rearrange("(oc p) i -> p oc i", p=P)
                    )
                    amax_p = stats.tile([P, 1], F32, tag=f"amax_p_{nm}")
                    nc.vector.tensor_reduce(
                        amax_p[:], w_sb[:], XY_AXIS, MAX, apply_absolute_value=True
                    )
                    amax = stats.tile([P, 1], F32, tag=f"amax_{nm}")
                    nc.gpsimd.partition_all_reduce(
                        amax[:], amax_p[:], channels=P, reduce_op=bass_isa.ReduceOp.max
                    )
                    nc.vector.tensor_scalar(
                        w_scale[nm][:], amax[:], 1.0 / 127.0, None, MULT
                    )
                    inv_s = stats.tile([P, 1], F32, tag=f"invs_{nm}")
                    nc.vector.reciprocal(inv_s[:], amax[:])
                    nc.vector.tensor_scalar(inv_s[:], inv_s[:], 127.0, None, MULT)
                    nc.vector.tensor_scalar(
                        w_sb[:], w_sb[:], inv_s[:], C_MAGIC, MULT, ADD
                    )
                    nc.vector.tensor_scalar(w_sb[:], w_sb[:], C_MAGIC, None, SUB)
                    # ---- transpose W' ints -> [i, o] ----
                    wT = wTp.tile([P, OC, H], F32R, tag="wT")
                    _transpose_to(nc, trp, wT, w_sb, ident[:])
                    if nm == "o":
                        nc.any.tensor_copy(out=woT[:], in_=wT[:])
                        continue
                    # swi = s_w / s_act  (fold weight scale + act quant step)
                    swi = stats.tile([P, 1], F32, tag=f"swi_{nm}")
                    nc.vector.tensor_scalar(
                        swi[:], w_scale[nm][:], INV_S_ACT, None, MULT
                    )
                    if nm in ("q", "k"):
                        out_int = qT if nm == "q" else kT
                        # GEMM -> [o_p, t_f]
                        for oc in range(OC):
                            for th in range(2):
                                ps = mmp.tile([P, 512], F32, tag="proj_ps")
                                for ic in range(OC):
                                    nc.tensor.matmul(
                                        ps[:],
                                        wT[:, ic, oc * P : (oc + 1) * P],
                                        xT[:, ic, th * 512 : (th + 1) * 512],
                                        start=(ic == 0),
                                        stop=(ic == OC - 1),
                                    )
                                t1 = epi.tile([P, 512], F32, tag="prj_t1")
                                nc.vector.tensor_scalar(
                                    t1[:],
                                    ps[:],
                                    swi[:],
                                    b50[nm][:, oc : oc + 1],
                                    MULT,
                                    ADD,
                                )
                                nc.vector.tensor_scalar(
                                    t1[:], t1[:], 127.0, -127.0, MIN, MAX
                                )
                                nc.vector.tensor_scalar(
                                    out_int[:, oc, th * 512 : (th + 1) * 512],
                                    t1[:],
                                    C_MAGIC,
                                    C_MAGIC,
                                    ADD,
                                    SUB,
                                )
                    else:  # v: GEMM -> [t_p, o_f]
                        for tcc in range(TC):
                            for oh in range(2):
                                ps = mmp.tile([P, 512], F32, tag="proj_ps")
                                for ic in range(OC):
                                    nc.tensor.matmul(
                                        ps[:],
                                        xT[:, ic, tcc * P : (tcc + 1) * P],
                                        wT[:, ic, oh * 512 : (oh + 1) * 512],
                                        start=(ic == 0),
                                        stop=(ic == OC - 1),
                                    )
                                t1 = epi.tile([P, 512], F32, tag="prj_t1")
                                nc.vector.scalar_tensor_tensor(
                                    t1[:],
                                    ps[:],
                                    swi[:],
                                    bv50_rep[:, oh * 512 : (oh + 1) * 512],
                                    MULT,
                                    ADD,
                                )
                                nc.vector.tensor_scalar(
                                    t1[:], t1[:], 127.0, -127.0, MIN, MAX
                                )
                                nc.vector.tensor_scalar(
                                    v_int[:, tcc, oh * 512 : (oh + 1) * 512],
                                    t1[:],
                                    C_MAGIC,
                                    C_MAGIC,
                                    ADD,
                                    SUB,
                                )

            # -------- per-head attention ------------------------------
            with (
                tc.tile_pool(name="rr_dram", bufs=2, space="DRAM") as rrdp,
                tc.tile_pool(name="l1_psum", bufs=2, space="PSUM") as p1pool,
                tc.tile_pool(name="l2_psum", bufs=1, space="PSUM") as p2pool,
                tc.tile_pool(name="ctx_psum", bufs=1, space="PSUM") as pcpool,
                tc.tile_pool(name="l1", bufs=3) as l1pool,
                tc.tile_pool(name="l2", bufs=3) as l2pool,
                tc.tile_pool(name="hstat", bufs=3) as hstat,
                tc.tile_pool(name="ctxepi", bufs=2) as ctxepi,
            ):
                ctx_ps = None
                for h in range(NH):
                    hp = h % 2
                    oc_h = h // 2
                    plo = D * hp
                    qT_h = qT[plo : plo + D, oc_h, :]  # [64, 1024] ints
                    kT_h = kT[plo : plo + D, oc_h, :]

                    rs = hstat.tile([P, TC], F32, tag="rs")
                    rr = hstat.tile([P, TC], F32, tag="rr")
                    rr50 = hstat.tile([P, TC], F32, tag="rr50")
                    rrT = hstat.tile([1, S], F32, tag="rrT")

                    # ---- layout 1: scores [q, k] ----
                    for qc in range(TC):
                        ps1 = p1pool.tile([P, S], F32, tag="ps1")
                        for kh in range(2):
                            nc.tensor.matmul(
                                ps1[:, kh * 512 : (kh + 1) * 512],
                                qT_h[:, qc * P : (qc + 1) * P],
                                kT_h[:, kh * 512 : (kh + 1) * 512],
                                start=True,
                                stop=not mask_nonzero,
                            )
                            if mask_nonzero:
                                nc.tensor.matmul(
                                    ps1[:, kh * 512 : (kh + 1) * 512],
                                    ones_row[0:1, qc * P : (qc + 1) * P],
                                    maskc[0:1, kh * 512 : (kh + 1) * 512],
                                    start=False,
                                    stop=True,
                                    skip_group_check=True,
                                )
                        sc_t = l1pool.tile([P, S], F32, tag="sc")
                        nc.scalar.activation(sc_t[:], ps1[:], Copy, scale=C_QK)
                        nc.sync.dma_start(
                            out_scores[h, qc * P : (qc + 1) * P, :], sc_t[:]
                        )
                        un_t = l1pool.tile([P, S], F32, tag="un")
                        nc.scalar.activation(
                            un_t[:],
                            ps1[:],
                            Exp,
                            scale=C_QK,
                            accum_out=rs[:, qc : qc + 1],
                        )
                        nc.vector.reciprocal(rr[:, qc : qc + 1], rs[:, qc : qc + 1])
                        pr_t = l1pool.tile([P, S], F32, tag="pr")
                        nc.vector.tensor_scalar(
                            pr_t[:], un_t[:], rr[:, qc : qc + 1], None, MULT
                        )
                        nc.sync.dma_start(
                            out_probs[h, qc * P : (qc + 1) * P, :], pr_t[:]
                        )

                    nc.vector.tensor_scalar(rr50[:], rr[:], INV_S_ACT, None, MULT)
                    # scatter [128, TC] -> flat [S] (t = qc*128 + p) via DRAM
                    rr_d = rrdp.tile([S], F32, tag="rr_d")
                    with nc.allow_non_contiguous_dma(reason="tiny per-head stats"):
                        nc.sync.dma_start(
                            rr_d[:].rearrange("(c p) -> p c", p=P), rr50[:, :]
                        )
                    nc.sync.dma_start(rrT[:], rr_d[None, :])
                    rrT_rep = hstat.tile([P, S], F32, tag="rrT_rep")
                    nc.gpsimd.partition_broadcast(rrT_rep[:], rrT[0:1, :])

                    # ---- layout 2: scores^T [k, q] feeding PV ----
                    if hp == 0:
                        ctx_ps = pcpool.tile([P, S], F32, tag="ctx_ps")
                    for kc in range(TC):
                        ps2 = p2pool.tile([P, S], F32, tag="ps2")
                        for qh in range(2):
                            nc.tensor.matmul(
                                ps2[:, qh * 512 : (qh + 1) * 512],
                                kT_h[:, kc * P : (kc + 1) * P],
                                qT_h[:, qh * 512 : (qh + 1) * 512],
                                start=True,
                                stop=True,
                            )
                        un2_t = l2pool.tile([P, S], F32, tag="un2")
                        if mask_nonzero:
                            nc.scalar.activation(
                                un2_t[:],
                                ps2[:],
                                Exp,
                                scale=C_QK,
                                bias=mask_pp[:, kc : kc + 1],
                            )
                        else:
                            nc.scalar.activation(un2_t[:], ps2[:], Exp, scale=C_QK)
                        t2_t = l2pool.tile([P, S], F32, tag="t2")
                        nc.gpsimd.tensor_tensor(t2_t[:], un2_t[:], rrT_rep[:], MULT)
                        aT_t = l2pool.tile([P, S], BF16, tag="aT")
                        nc.vector.tensor_scalar(
                            aT_t[:], t2_t[:], C_MAGIC, C_MAGIC, ADD, SUB
                        )
                        for qh in range(2):
                            nc.tensor.matmul(
                                ctx_ps[plo : plo + D, qh * 512 : (qh + 1) * 512],
                                v_int[:, kc, h * D : (h + 1) * D],
                                aT_t[:, qh * 512 : (qh + 1) * 512],
                                start=(kc == 0),
                                stop=(kc == TC - 1),
                                skip_group_check=True,
                            )

                    if hp == 1:
                        # fake-quant ctx^T pair -> a2[:, oc_h, :] (ints, bf16)
                        c_pv = float(np.float32(S_ACT * S_ACT) * np.float32(INV_S_ACT))
                        e1 = ctxepi.tile([P, S], F32, tag="ce1")
                        nc.vector.tensor_scalar(
                            e1[:], ctx_ps[:], c_pv, 127.0, MULT, MIN
                        )
                        nc.vector.tensor_scalar(
                            e1[:], e1[:], -127.0, C_MAGIC, MAX, ADD
                        )
                        nc.vector.tensor_scalar(
                            a2[:, oc_h, :], e1[:], C_MAGIC, None, SUB
                        )

            # -------- output projection + residual + LayerNorm --------
            with (
                tc.tile_pool(name="o_psum", bufs=2, space="PSUM") as opool,
                tc.tile_pool(name="fin", bufs=3) as fin,
                tc.tile_pool(name="rows", bufs=1) as rows,
                tc.tile_pool(name="lnstat", bufs=2) as lnstat,
            ):
                bo_rep = rows.tile([P, H], F32, tag="bo_rep")
                nc.gpsimd.partition_broadcast(bo_rep[:], bo_row[0:1, :])
                g_rep = rows.tile([P, H], F32, tag="g_rep")
                nc.gpsimd.partition_broadcast(g_rep[:], g_row[0:1, :])
                beta_rep = rows.tile([P, H], F32, tag="beta_rep")
                nc.gpsimd.partition_broadcast(beta_rep[:], beta_row[0:1, :])
                c_o = lnstat.tile([P, 1], F32, tag="c_o")  # s_ctx * s_wo
                nc.vector.tensor_scalar(
                    c_o[:], w_scale["o"][:], float(S_ACT), None, MULT
                )
                eps_t = lnstat.tile([P, 1], F32, tag="eps_t")
                nc.vector.memset(eps_t[:], LN_EPS)
                for tcc in range(TC):
                    ps = opool.tile([P, H], F32, tag="ops")
                    for oh in range(2):
                        for hc in range(OC):
                            nc.tensor.matmul(
                                ps[:, oh * 512 : (oh + 1) * 512],
                                a2[:, hc, tcc * P : (tcc + 1) * P],
                                woT[:, hc, oh * 512 : (oh + 1) * 512],
                                start=(hc == 0),
                                stop=(hc == OC - 1),
                            )
                    hid_t = fin.tile([P, H], F32, tag="hid_t")
                    nc.sync.dma_start(hid_t[:], hid[tcc * P : (tcc + 1) * P, :])
                    nc.vector.tensor_tensor(hid_t[:], hid_t[:], bo_rep[:], ADD)
                    x_t = fin.tile([P, H], F32, tag="x_t")
                    nc.vector.scalar_tensor_tensor(
                        x_t[:], ps[:], c_o[:], hid_t[:], MULT, ADD
                    )
                    # LayerNorm over the free (H) dim
                    xsum = lnstat.tile([P, 1], F32, tag="xsum")
                    nc.vector.tensor_reduce(xsum[:], x_t[:], XY_AXIS, ADD)
                    mu = lnstat.tile([P, 1], F32, tag="mu")
                    nc.vector.tensor_scalar(mu[:], xsum[:], 1.0 / H, None, MULT)
                    xc_t = fin.tile([P, H], F32, tag="xc_t")
                    varsum = lnstat.tile([P, 1], F32, tag="varsum")
                    nc.vector.tensor_scalar(xc_t[:], x_t[:], mu[:], None, SUB)
                    # square into the spent hid_t slot; only accum_out matters
                    nc.vector.scalar_tensor_tensor(
                        hid_t[:], xc_t[:], 1.0, xc_t[:], MULT, MULT,
                        accum_out=varsum[:],
                    )
                    std = lnstat.tile([P, 1], F32, tag="std")
                    nc.scalar.activation(
                        std[:], varsum[:], Sqrt, scale=1.0 / H, bias=eps_t[:]
                    )
                    rstd = lnstat.tile([P, 1], F32, tag="rstd")
                    nc.vector.reciprocal(rstd[:], std[:])
                    nc.vector.tensor_scalar(xc_t[:], xc_t[:], rstd[:], None, MULT)
                    nc.vector.tensor_tensor(xc_t[:], xc_t[:], g_rep[:], MULT)
                    nc.vector.tensor_tensor(xc_t[:], xc_t[:], beta_rep[:], ADD)
                    nc.sync.dma_start(out_attn[tcc * P : (tcc + 1) * P, :], xc_t[:])

    nc.compile()
    return nc


_CACHE = {}


def _get_nc(mask_nonzero):
    key = bool(mask_nonzero)
    if key not in _CACHE:
        _CACHE[key] = build(key)
    return _CACHE[key]


def kernel(
    hidden_states,
    attention_mask,
    Wq,
    bq,
    Wk,
    bk,
    Wv,
    bv,
    Wo,
    bo,
    ln_gamma,
    ln_beta,
    trace=False,
    **trace_kwargs,
):
    hidden_states = np.asarray(hidden_states, dtype=np.float32)
    attention_mask = np.asarray(attention_mask, dtype=np.float32)
    B = hidden_states.shape[0]
    assert B == 8 and hidden_states.shape[1:] == (S, H)

    mask_nonzero = bool(np.any(attention_mask != 0.0))
    nc = _get_nc(mask_nonzero)

    shared = {
        "Wq": np.ascontiguousarray(Wq, np.float32),
        "Wk": np.ascontiguousarray(Wk, np.float32),
        "Wv": np.ascontiguousarray(Wv, np.float32),
        "Wo": np.ascontiguousarray(Wo, np.float32),
        "bq": np.ascontiguousarray(bq, np.float32),
        "bk": np.ascontiguousarray(bk, np.float32),
        "bv": np.ascontiguousarray(bv, np.float32),
        "bo": np.ascontiguousarray(bo, np.float32),
        "ln_gamma": np.ascontiguousarray(ln_gamma, np.float32),
        "ln_beta": np.ascontiguousarray(ln_beta, np.float32),
    }
    in_maps = []
    for b in range(B):
        m = dict(shared)
        m["hidden"] = np.ascontiguousarray(hidden_states[b])
        m["mask"] = np.ascontiguousarray(
            np.broadcast_to(attention_mask[b], (1, 1, S)).reshape(S)
        )
        in_maps.append(m)

    res = run_bass_kernel_spmd(
        nc, in_maps, core_ids=list(range(8)), trace=trace, **trace_kwargs
    )
    attn = np.stack([res.results[b]["attn_out"] for b in range(B)])
    scores = np.stack([res.results[b]["scores"] for b in range(B)])
    probs = np.stack([res.results[b]["probs"] for b in range(B)])
    kernel.last_results = res
    return attn, scores, probs


# revision 13
# speedup vs baseline: 2.6328x; 1.0690x over previous
"""BertAttention (quantized, eval) Trainium2 kernel.

Data-parallel over batch: 8 batch elements -> 8 NeuronCores, one full
attention layer per core, no collectives.  Returns the same tuple as the
reference: (attention_output, scores, probs).

Key ideas:
- Every fake-quantized tensor (weights, q, k, v, probs, ctx) is
  round(x/s) * s with round(x/s) an integer in [-127, 127].  We carry the
  integers (exact in bf16) through the TensorEngine, so QK^T, PV and the
  output projection are exact integer matmuls accumulated in f32 PSUM
  (max |sum| < 2^24); the f32 scales are applied in the epilogues.
- round-half-to-even == (x + 1.5*2^23) - 1.5*2^23 in f32, one 2-op
  tensor_scalar instruction (matches jnp.round for |x| <= 2^22).
- scores are computed in both [q,k] layout (scores/probs outputs, softmax
  row sums via the ACT engine's accum_out) and [k,q] layout (so the PV
  matmul gets probs^T without transposing the 64MB probs tensor).  Both
  layouts reuse the same Q^T / K^T integer operands.
"""

import sys

sys.path.insert(0, "/opt/trn_rl_repo")

import numpy as np

import concourse.bass as bass
import concourse.bass_isa as bass_isa
import concourse.tile as tile
from concourse import bacc, mybir
from concourse.bass_utils import run_bass_kernel_spmd
from concourse.masks import make_identity

H = 1024
S = 1024
NH = 16
D = 64
P = 128
OC = H // P  # 8 chunks of 128 output channels
TC = S // P  # 8 chunks of 128 tokens
F32 = mybir.dt.float32
F32R = mybir.dt.float32r
BF16 = mybir.dt.bfloat16

CLIP = np.float32(2.5)
N_LEVELS = np.float32(127.0)
S_ACT = np.float32(CLIP / N_LEVELS)  # activation quant step
INV_S_ACT = float(np.float32(1.0) / S_ACT)  # ~50.8
C_QK = float(np.float32(S_ACT * S_ACT) / np.float32(8.0))  # sqrt(D) == 8
C_MAGIC = 12582912.0  # 1.5 * 2**23: f32 round-to-nearest-even bias
LN_EPS = 1e-12

ADD = mybir.AluOpType.add
SUB = mybir.AluOpType.subtract
MULT = mybir.AluOpType.mult
MIN = mybir.AluOpType.min
MAX = mybir.AluOpType.max
BYPASS = mybir.AluOpType.bypass
Exp = mybir.ActivationFunctionType.Exp
Copy = mybir.ActivationFunctionType.Copy
Sqrt = mybir.ActivationFunctionType.Sqrt
XY_AXIS = mybir.AxisListType.XY


def _transpose_to(nc, psum_pool, dst, src, ident):
    """PE-transpose [P, C, J*P] -> [P, J, C*P]: each 128x128 tile
    src[:, c, j*P:+P] lands transposed in dst[:, j, c*P:+P]."""
    n_c = src.shape[1]
    n_j = src.shape[2] // P
    for c in range(n_c):
        for j in range(n_j):
            pt = psum_pool.tile([P, P], F32, tag="tr_psum")
            nc.tensor.transpose(pt[:], src[:, c, j * P : (j + 1) * P], ident)
            nc.any.tensor_copy(out=dst[:, j, c * P : (c + 1) * P], in_=pt[:])


def build(mask_nonzero: bool):
    nc = bacc.Bacc("TRN2", target_bir_lowering=False, debug=False, num_devices=8)

    hid = nc.dram_tensor("hidden", [S, H], F32, kind="ExternalInput").ap()
    msk = nc.dram_tensor("mask", [S], F32, kind="ExternalInput").ap()
    w_dr = {}
    b_dr = {}
    for nm, w_name, b_name in (
        ("q", "Wq", "bq"),
        ("k", "Wk", "bk"),
        ("v", "Wv", "bv"),
        ("o", "Wo", "bo"),
    ):
        w_dr[nm] = nc.dram_tensor(w_name, [H, H], F32, kind="ExternalInput").ap()
        b_dr[nm] = nc.dram_tensor(b_name, [H], F32, kind="ExternalInput").ap()
    g_ln = nc.dram_tensor("ln_gamma", [H], F32, kind="ExternalInput").ap()
    beta_ln = nc.dram_tensor("ln_beta", [H], F32, kind="ExternalInput").ap()

    out_attn = nc.dram_tensor("attn_out", [S, H], F32, kind="ExternalOutput").ap()
    out_scores = nc.dram_tensor("scores", [NH, S, S], F32, kind="ExternalOutput").ap()
    out_probs = nc.dram_tensor("probs", [NH, S, S], F32, kind="ExternalOutput").ap()

    with tile.TileContext(nc) as tc:
        with tc.tile_pool(name="pers", bufs=1) as pers:
            # -------- persistent tensors (span several phases) --------
            qT = pers.tile([P, OC, S], BF16, tag="qT")  # q^T ints [o, t]
            kT = pers.tile([P, OC, S], BF16, tag="kT")
            v_int = pers.tile([P, TC, H], BF16, tag="v_int")  # v ints [t, o]
            a2 = pers.tile([P, OC, S], BF16, tag="a2")  # ctx_q^T ints [H, t]
            woT = pers.tile([P, OC, H], BF16, tag="woT")  # Wo'^T ints [i, o]
            ident = pers.tile([P, P], F32, tag="ident")
            make_identity(nc, ident[:])
            # per-o-channel bias carriers ([o%128, oc]) pre-scaled by 1/s
            b50 = {}
            for nm in ("q", "k"):
                braw = pers.tile([P, OC], F32, tag=f"braw_{nm}")
                nc.sync.dma_start(braw[:], b_dr[nm].rearrange("(oc p) -> p oc", p=P))
                b50[nm] = pers.tile([P, OC], F32, tag=f"b50_{nm}", name=f"b50_{nm}")
                nc.vector.tensor_scalar(b50[nm][:], braw[:], INV_S_ACT, None, MULT)
            bv_row = pers.tile([1, H], F32, tag="bv_row")
            nc.sync.dma_start(bv_row[:], b_dr["v"][None, :])
            bv50 = pers.tile([1, H], F32, tag="bv50")
            nc.vector.tensor_scalar(bv50[:], bv_row[:], INV_S_ACT, None, MULT)
            bv50_rep = pers.tile([P, H], F32, tag="bv50_rep")
            nc.gpsimd.partition_broadcast(bv50_rep[:], bv50[0:1, :])
            bo_row = pers.tile([1, H], F32, tag="bo_row")
            nc.sync.dma_start(bo_row[:], b_dr["o"][None, :])
            g_row = pers.tile([1, H], F32, tag="g_row")
            nc.sync.dma_start(g_row[:], g_ln[None, :])
            beta_row = pers.tile([1, H], F32, tag="beta_row")
            nc.sync.dma_start(beta_row[:], beta_ln[None, :])
            if mask_nonzero:
                mask_pp = pers.tile([P, TC], F32, tag="mask_pp")  # [k%128, kc]
                nc.sync.dma_start(mask_pp[:], msk.rearrange("(kc p) -> p kc", p=P))
                mask_row = pers.tile([1, S], F32, tag="mask_row")
                nc.sync.dma_start(mask_row[:], msk[None, :])
                maskc = pers.tile([1, S], F32, tag="maskc")  # mask / C_QK
                nc.vector.tensor_scalar(maskc[:], mask_row[:], 1.0 / C_QK, None, MULT)
                ones_row = pers.tile([1, S], F32, tag="ones_row")
                nc.vector.memset(ones_row[:], 1.0)
            # weight scales (s_w = absmax/127), one [P,1] carrier per weight
            w_scale = {
                nm: pers.tile([P, 1], F32, tag=f"ws_{nm}", name=f"ws_{nm}")
                for nm in w_dr
            }

            # -------- phase A/B: x^T, quantized weights, projections --
            with (
                tc.tile_pool(name="xT_pool", bufs=1) as xTp,
                tc.tile_pool(name="tr_psum", bufs=4, space="PSUM") as trp,
                tc.tile_pool(name="mm_psum", bufs=4, space="PSUM") as mmp,
                tc.tile_pool(name="epi", bufs=2) as epi,
                tc.tile_pool(name="wstage", bufs=1) as wsp,
                tc.tile_pool(name="wT_pool", bufs=1) as wTp,
                tc.tile_pool(name="stats", bufs=1) as stats,
            ):
                xT = xTp.tile([P, OC, S], F32R, tag="xT")  # x^T [i, t]
                x_sb = wsp.tile([P, TC, H], F32, tag="w_sb", name="x_sb")
                nc.sync.dma_start(x_sb[:], hid.rearrange("(t p) i -> p t i", p=P))
                _transpose_to(nc, trp, xT, x_sb, ident[:])

                for nm in ("o", "v", "q", "k"):
                    # ---- load + per-tensor quantize (in place) ----
                    w_sb = wsp.tile([P, OC, H], F32, tag="w_sb")
                    nc.sync.dma_start(
                        w_sb[:], w_dr[nm].rearrange("(oc p) i -> p oc i", p=P)
                    )
                    amax_p = stats.tile([P, 1], F32, tag=f"amax_p_{nm}")
                    nc.vector.tensor_reduce(
                        amax_p[:], w_sb[:], XY_AXIS, MAX, apply_absolute_value=True
                    )
                    amax = stats.tile([P, 1], F32, tag=f"amax_{nm}")
                    nc.gpsimd.partition_all_reduce(
                        amax[:], amax_p[:], channels=P, reduce_op=bass_isa.ReduceOp.max
                    )
                    nc.vector.tensor_scalar(
                        w_scale[nm][:], amax[:], 1.0 / 127.0, None, MULT
                    )
                    inv_s = stats.tile([P, 1], F32, tag=f"invs_{nm}")
                    nc.vector.reciprocal(inv_s[:], amax[:])
                    nc.vector.tensor_scalar(inv_s[:], inv_s[:], 127.0, None, MULT)
                    nc.vector.tensor_scalar(
                        w_sb[:], w_sb[:], inv_s[:], C_MAGIC, MULT, ADD
                    )
                    nc.vector.tensor_scalar(w_sb[:], w_sb[:], C_MAGIC, None, SUB)
                    # ---- transpose W' ints -> [i, o] ----
                    wT = wTp.tile([P, OC, H], F32R, tag="wT")
                    _transpose_to(nc, trp, wT, w_sb, ident[:])
                    if nm == "o":
                        nc.any.tensor_copy(out=woT[:], in_=wT[:])
                        continue
                    # swi = s_w / s_act  (fold weight scale + act quant step)
                    swi = stats.tile([P, 1], F32, tag=f"swi_{nm}")
                    nc.vector.tensor_scalar(
                        swi[:], w_scale[nm][:], INV_S_ACT, None, MULT
                    )
                    if nm in ("q", "k"):
                        out_int = qT if nm == "q" else kT
                        # GEMM -> [o_p, t_f]
                        for oc in range(OC):
                            for th in range(2):
                                ps = mmp.tile([P, 512], F32, tag="proj_ps")
                                for ic in range(OC):
                                    nc.tensor.matmul(
                                        ps[:],
                                        wT[:, ic, oc * P : (oc + 1) * P],
                                        xT[:, ic, th * 512 : (th + 1) * 512],
                                        start=(ic == 0),
                                        stop=(ic == OC - 1),
                                    )
                                t1 = epi.tile([P, 512], F32, tag="prj_t1")
                                nc.vector.tensor_scalar(
                                    t1[:],
                                    ps[:],
                                    swi[:],
                                    b50[nm][:, oc : oc + 1],
                                    MULT,
                                    ADD,
                                )
                                nc.vector.tensor_scalar(
                                    t1[:], t1[:], 127.0, -127.0, MIN, MAX
                                )
                                nc.vector.tensor_scalar(
                                    out_int[:, oc, th * 512 : (th + 1) * 512],
                                    t1[:],
                                    C_MAGIC,
                                    C_MAGIC,
                                    ADD,
                                    SUB,
                                )
                    else:  # v: GEMM -> [t_p, o_f]
                        for tcc in range(TC):
                            for oh in range(2):
                                ps = mmp.tile([P, 512], F32, tag="proj_ps")
                                for ic in range(OC):
                                    nc.tensor.matmul(
                                        ps[:],
                                        xT[:, ic, tcc * P : (tcc + 1) * P],
                                        wT[:, ic, oh * 512 : (oh + 1) * 512],
                                        start=(ic == 0),
                                        stop=(ic == OC - 1),
                                    )
                                t1 = epi.tile([P, 512], F32, tag="prj_t1")
                                nc.vector.scalar_tensor_tensor(
                                    t1[:],
                                    ps[:],
                                    swi[:],
                                    bv50_rep[:, oh * 512 : (oh + 1) * 512],
                                    MULT,
                                    ADD,
                                )
                                nc.vector.tensor_scalar(
                                    t1[:], t1[:], 127.0, -127.0, MIN, MAX
                                )
                                nc.vector.tensor_scalar(
                                    v_int[:, tcc, oh * 512 : (oh + 1) * 512],
                                    t1[:],
                                    C_MAGIC,
                                    C_MAGIC,
                                    ADD,
                                    SUB,
                                )

            # -------- per-head attention ------------------------------
            with (
                tc.tile_pool(name="rr_dram", bufs=2, space="DRAM") as rrdp,
                tc.tile_pool(name="l1_psum", bufs=2, space="PSUM") as p1pool,
                tc.tile_pool(name="l2_psum", bufs=1, space="PSUM") as p2pool,
                tc.tile_pool(name="ctx_psum", bufs=1, space="PSUM") as pcpool,
                tc.tile_pool(name="l1", bufs=3) as l1pool,
                tc.tile_pool(name="l2", bufs=3) as l2pool,
                tc.tile_pool(name="hstat", bufs=3) as hstat,
                tc.tile_pool(name="ctxepi", bufs=2) as ctxepi,
            ):
                ctx_tiles = {}
                head_state = {}

                def emit_l1(h):
                    hp = h % 2
                    oc_h = h // 2
                    plo = D * hp
                    qT_h = qT[plo : plo + D, oc_h, :]  # [64, 1024] ints
                    kT_h = kT[plo : plo + D, oc_h, :]

                    rs = hstat.tile([P, TC], F32, tag="rs", name="rs")
                    rr = hstat.tile([P, TC], F32, tag="rr", name="rr")
                    rr50 = hstat.tile([P, TC], F32, tag="rr50", name="rr50")
                    rrT = hstat.tile([1, S], F32, tag="rrT", name="rrT")

                    # ---- layout 1: scores [q, k] ----
                    for qc in range(TC):
                        ps1 = p1pool.tile([P, S], F32, tag="ps1")
                        for kh in range(2):
                            nc.tensor.matmul(
                                ps1[:, kh * 512 : (kh + 1) * 512],
                                qT_h[:, qc * P : (qc + 1) * P],
                                kT_h[:, kh * 512 : (kh + 1) * 512],
                                start=True,
                                stop=not mask_nonzero,
                            )
                            if mask_nonzero:
                                nc.tensor.matmul(
                                    ps1[:, kh * 512 : (kh + 1) * 512],
                                    ones_row[0:1, qc * P : (qc + 1) * P],
                                    maskc[0:1, kh * 512 : (kh + 1) * 512],
                                    start=False,
                                    stop=True,
                                    skip_group_check=True,
                                )
                        sc_t = l1pool.tile([P, S], F32, tag="sc")
                        nc.scalar.activation(sc_t[:], ps1[:], Copy, scale=C_QK)
                        nc.sync.dma_start(
                            out_scores[h, qc * P : (qc + 1) * P, :], sc_t[:]
                        )
                        un_t = l1pool.tile([P, S], F32, tag="un")
                        nc.scalar.activation(
                            un_t[:],
                            ps1[:],
                            Exp,
                            scale=C_QK,
                            accum_out=rs[:, qc : qc + 1],
                        )
                        nc.vector.reciprocal(rr[:, qc : qc + 1], rs[:, qc : qc + 1])
                        pr_t = l1pool.tile([P, S], F32, tag="pr")
                        nc.vector.tensor_scalar(
                            pr_t[:], un_t[:], rr[:, qc : qc + 1], None, MULT
                        )
                        nc.sync.dma_start(
                            out_probs[h, qc * P : (qc + 1) * P, :], pr_t[:]
                        )

                    nc.vector.tensor_scalar(rr50[:], rr[:], INV_S_ACT, None, MULT)
                    # scatter [128, TC] -> flat [S] (t = qc*128 + p) via DRAM
                    rr_d = rrdp.tile([S], F32, tag="rr_d")
                    with nc.allow_non_contiguous_dma(reason="tiny per-head stats"):
                        nc.sync.dma_start(
                            rr_d[:].rearrange("(c p) -> p c", p=P), rr50[:, :]
                        )
                    nc.sync.dma_start(rrT[:], rr_d[None, :])
                    rrT_rep = hstat.tile([P, S], F32, tag="rrT_rep", name="rrT_rep")
                    nc.gpsimd.partition_broadcast(rrT_rep[:], rrT[0:1, :])
                    head_state[h] = rrT_rep

                def emit_l2(h):
                    hp = h % 2
                    oc_h = h // 2
                    plo = D * hp
                    qT_h = qT[plo : plo + D, oc_h, :]
                    kT_h = kT[plo : plo + D, oc_h, :]
                    rrT_rep = head_state.pop(h)

                    # ---- layout 2: scores^T [k, q] feeding PV ----
                    if hp == 0:
                        ctx_tiles[oc_h] = pcpool.tile(
                            [P, S], F32, tag="ctx_ps", name="ctx_ps"
                        )
                    ctx_ps = ctx_tiles[oc_h]
                    for kc in range(TC):
                        ps2 = p2pool.tile([P, S], F32, tag="ps2")
                        for qh in range(2):
                            nc.tensor.matmul(
                                ps2[:, qh * 512 : (qh + 1) * 512],
                                kT_h[:, kc * P : (kc + 1) * P],
                                qT_h[:, qh * 512 : (qh + 1) * 512],
                                start=True,
                                stop=True,
                            )
                        un2_t = l2pool.tile([P, S], F32, tag="un2")
                        if mask_nonzero:
                            nc.scalar.activation(
                                un2_t[:],
                                ps2[:],
                                Exp,
                                scale=C_QK,
                                bias=mask_pp[:, kc : kc + 1],
                            )
                        else:
                            nc.scalar.activation(un2_t[:], ps2[:], Exp, scale=C_QK)
                        t2_t = l2pool.tile([P, S], F32, tag="t2")
                        nc.gpsimd.tensor_tensor(t2_t[:], un2_t[:], rrT_rep[:], MULT)
                        aT_t = l2pool.tile([P, S], BF16, tag="aT")
                        nc.vector.tensor_scalar(
                            aT_t[:], t2_t[:], C_MAGIC, C_MAGIC, ADD, SUB
                        )
                        for qh in range(2):
                            nc.tensor.matmul(
                                ctx_ps[plo : plo + D, qh * 512 : (qh + 1) * 512],
                                v_int[:, kc, h * D : (h + 1) * D],
                                aT_t[:, qh * 512 : (qh + 1) * 512],
                                start=(kc == 0),
                                stop=(kc == TC - 1),
                                skip_group_check=True,
                            )

                    if hp == 1:
                        # fake-quant ctx^T pair -> a2[:, oc_h, :] (ints, bf16)
                        ctx_tiles.pop(oc_h)
                        c_pv = float(np.float32(S_ACT * S_ACT) * np.float32(INV_S_ACT))
                        e1 = ctxepi.tile([P, S], F32, tag="ce1", name="e1")
                        nc.vector.tensor_scalar(
                            e1[:], ctx_ps[:], c_pv, 127.0, MULT, MIN
                        )
                        nc.vector.tensor_scalar(
                            e1[:], e1[:], -127.0, C_MAGIC, MAX, ADD
                        )
                        nc.vector.tensor_scalar(
                            a2[:, oc_h, :], e1[:], C_MAGIC, None, SUB
                        )

                # software pipeline: L1(h) overlaps L2(h-1) in every
                # engine's program order
                emit_l1(0)
                for h in range(1, NH):
                    emit_l1(h)
                    emit_l2(h - 1)
                emit_l2(NH - 1)

            # -------- output projection + residual + LayerNorm --------
            with (
                tc.tile_pool(name="o_psum", bufs=2, space="PSUM") as opool,
                tc.tile_pool(name="fin", bufs=3) as fin,
                tc.tile_pool(name="rows", bufs=1) as rows,
                tc.tile_pool(name="lnstat", bufs=2) as lnstat,
            ):
                bo_rep = rows.tile([P, H], F32, tag="bo_rep")
                nc.gpsimd.partition_broadcast(bo_rep[:], bo_row[0:1, :])
                g_rep = rows.tile([P, H], F32, tag="g_rep")
                nc.gpsimd.partition_broadcast(g_rep[:], g_row[0:1, :])
                beta_rep = rows.tile([P, H], F32, tag="beta_rep")
                nc.gpsimd.partition_broadcast(beta_rep[:], beta_row[0:1, :])
                c_o = lnstat.tile([P, 1], F32, tag="c_o")  # s_ctx * s_wo
                nc.vector.tensor_scalar(
                    c_o[:], w_scale["o"][:], float(S_ACT), None, MULT
                )
                eps_t = lnstat.tile([P, 1], F32, tag="eps_t")
                nc.vector.memset(eps_t[:], LN_EPS)
                for tcc in range(TC):
                    ps = opool.tile([P, H], F32, tag="ops")
                    for oh in range(2):
                        for hc in range(OC):
                            nc.tensor.matmul(
                                ps[:, oh * 512 : (oh + 1) * 512],
                                a2[:, hc, tcc * P : (tcc + 1) * P],
                                woT[:, hc, oh * 512 : (oh + 1) * 512],
                                start=(hc == 0),
                                stop=(hc == OC - 1),
                            )
                    hid_t = fin.tile([P, H], F32, tag="hid_t")
                    nc.sync.dma_start(hid_t[:], hid[tcc * P : (tcc + 1) * P, :])
                    nc.vector.tensor_tensor(hid_t[:], hid_t[:], bo_rep[:], ADD)
                    x_t = fin.tile([P, H], F32, tag="x_t")
                    nc.vector.scalar_tensor_tensor(
                        x_t[:], ps[:], c_o[:], hid_t[:], MULT, ADD
                    )
                    # LayerNorm over the free (H) dim
                    xsum = lnstat.tile([P, 1], F32, tag="xsum")
                    nc.vector.tensor_reduce(xsum[:], x_t[:], XY_AXIS, ADD)
                    mu = lnstat.tile([P, 1], F32, tag="mu")
                    nc.vector.tensor_scalar(mu[:], xsum[:], 1.0 / H, None, MULT)
                    xc_t = fin.tile([P, H], F32, tag="xc_t")
                    varsum = lnstat.tile([P, 1], F32, tag="varsum")
                    nc.vector.tensor_scalar(xc_t[:], x_t[:], mu[:], None, SUB)
                    # square into the spent hid_t slot; only accum_out matters
                    nc.vector.scalar_tensor_tensor(
                        hid_t[:], xc_t[:], 1.0, xc_t[:], MULT, MULT,
                        accum_out=varsum[:],
                    )
                    std = lnstat.tile([P, 1], F32, tag="std")
                    nc.scalar.activation(
                        std[:], varsum[:], Sqrt, scale=1.0 / H, bias=eps_t[:]
                    )
                    rstd = lnstat.tile([P, 1], F32, tag="rstd")
                    nc.vector.reciprocal(rstd[:], std[:])
                    nc.vector.tensor_scalar(xc_t[:], xc_t[:], rstd[:], None, MULT)
                    nc.vector.tensor_tensor(xc_t[:], xc_t[:], g_rep[:], MULT)
                    nc.vector.tensor_tensor(xc_t[:], xc_t[:], beta_rep[:], ADD)
                    nc.sync.dma_start(out_attn[tcc * P : (tcc + 1) * P, :], xc_t[:])

    nc.compile()
    return nc


_CACHE = {}


def _get_nc(mask_nonzero):
    key = bool(mask_nonzero)
    if key not in _CACHE:
        _CACHE[key] = build(key)
    return _CACHE[key]


def kernel(
    hidden_states,
    attention_mask,
    Wq,
    bq,
    Wk,
    bk,
    Wv,
    bv,
    Wo,
    bo,
    ln_gamma,
    ln_beta,
    trace=False,
    **trace_kwargs,
):
    hidden_states = np.asarray(hidden_states, dtype=np.float32)
    attention_mask = np.asarray(attention_mask, dtype=np.float32)
    B = hidden_states.shape[0]
    assert B == 8 and hidden_states.shape[1:] == (S, H)

    mask_nonzero = bool(np.any(attention_mask != 0.0))
    nc = _get_nc(mask_nonzero)

    shared = {
        "Wq": np.ascontiguousarray(Wq, np.float32),
        "Wk": np.ascontiguousarray(Wk, np.float32),
        "Wv": np.ascontiguousarray(Wv, np.float32),
        "Wo": np.ascontiguousarray(Wo, np.float32),
        "bq": np.ascontiguousarray(bq, np.float32),
        "bk": np.ascontiguousarray(bk, np.float32),
        "bv": np.ascontiguousarray(bv, np.float32),
        "bo": np.ascontiguousarray(bo, np.float32),
        "ln_gamma": np.ascontiguousarray(ln_gamma, np.float32),
        "ln_beta": np.ascontiguousarray(ln_beta, np.float32),
    }
    in_maps = []
    for b in range(B):
        m = dict(shared)
        m["hidden"] = np.ascontiguousarray(hidden_states[b])
        m["mask"] = np.ascontiguousarray(
            np.broadcast_to(attention_mask[b], (1, 1, S)).reshape(S)
        )
        in_maps.append(m)

    res = run_bass_kernel_spmd(
        nc, in_maps, core_ids=list(range(8)), trace=trace, **trace_kwargs
    )
    attn = np.stack([res.results[b]["attn_out"] for b in range(B)])
    scores = np.stack([res.results[b]["scores"] for b in range(B)])
    probs = np.stack([res.results[b]["probs"] for b in range(B)])
    kernel.last_results = res
    return attn, scores, probs


# revision 14
# speedup vs baseline: 2.6894x; 1.0215x over previous
"""BertAttention (quantized, eval) Trainium2 kernel.

Data-parallel over batch: 8 batch elements -> 8 NeuronCores, one full
attention layer per core, no collectives.  Returns the same tuple as the
reference: (attention_output, scores, probs).

Key ideas:
- Every fake-quantized tensor (weights, q, k, v, probs, ctx) is
  round(x/s) * s with round(x/s) an integer in [-127, 127].  We carry the
  integers (exact in bf16) through the TensorEngine, so QK^T, PV and the
  output projection are exact integer matmuls accumulated in f32 PSUM
  (max |sum| < 2^24); the f32 scales are applied in the epilogues.
- round-half-to-even == (x + 1.5*2^23) - 1.5*2^23 in f32, one 2-op
  tensor_scalar instruction (matches jnp.round for |x| <= 2^22).
- scores are computed in both [q,k] layout (scores/probs outputs, softmax
  row sums via the ACT engine's accum_out) and [k,q] layout (so the PV
  matmul gets probs^T without transposing the 64MB probs tensor).  Both
  layouts reuse the same Q^T / K^T integer operands.
"""

import os
import sys

sys.path.insert(0, "/opt/trn_rl_repo")
# 256B DRAM pages shatter DMA descriptors (2.5KB packets, ~225GB/s);
# 4KB pages let 4KB rows move as whole descriptors.
os.environ.setdefault("NEURON_SCRATCHPAD_PAGE_SIZE", "4096")

import numpy as np

import concourse.bass as bass
import concourse.bass_isa as bass_isa
import concourse.tile as tile
from concourse import bacc, mybir
from concourse.bass_utils import run_bass_kernel_spmd
from concourse.masks import make_identity

H = 1024
S = 1024
NH = 16
D = 64
P = 128
OC = H // P  # 8 chunks of 128 output channels
TC = S // P  # 8 chunks of 128 tokens
F32 = mybir.dt.float32
F32R = mybir.dt.float32r
BF16 = mybir.dt.bfloat16

CLIP = np.float32(2.5)
N_LEVELS = np.float32(127.0)
S_ACT = np.float32(CLIP / N_LEVELS)  # activation quant step
INV_S_ACT = float(np.float32(1.0) / S_ACT)  # ~50.8
C_QK = float(np.float32(S_ACT * S_ACT) / np.float32(8.0))  # sqrt(D) == 8
C_MAGIC = 12582912.0  # 1.5 * 2**23: f32 round-to-nearest-even bias
LN_EPS = 1e-12

ADD = mybir.AluOpType.add
SUB = mybir.AluOpType.subtract
MULT = mybir.AluOpType.mult
MIN = mybir.AluOpType.min
MAX = mybir.AluOpType.max
BYPASS = mybir.AluOpType.bypass
Exp = mybir.ActivationFunctionType.Exp
Copy = mybir.ActivationFunctionType.Copy
Sqrt = mybir.ActivationFunctionType.Sqrt
XY_AXIS = mybir.AxisListType.XY


def _transpose_to(nc, psum_pool, dst, src, ident):
    """PE-transpose [P, C, J*P] -> [P, J, C*P]: each 128x128 tile
    src[:, c, j*P:+P] lands transposed in dst[:, j, c*P:+P]."""
    n_c = src.shape[1]
    n_j = src.shape[2] // P
    for c in range(n_c):
        for j in range(n_j):
            pt = psum_pool.tile([P, P], F32, tag="tr_psum")
            nc.tensor.transpose(pt[:], src[:, c, j * P : (j + 1) * P], ident)
            nc.any.tensor_copy(out=dst[:, j, c * P : (c + 1) * P], in_=pt[:])


def build(mask_nonzero: bool):
    nc = bacc.Bacc("TRN2", target_bir_lowering=False, debug=False, num_devices=8)

    hid = nc.dram_tensor("hidden", [S, H], F32, kind="ExternalInput").ap()
    msk = nc.dram_tensor("mask", [S], F32, kind="ExternalInput").ap()
    w_dr = {}
    b_dr = {}
    for nm, w_name, b_name in (
        ("q", "Wq", "bq"),
        ("k", "Wk", "bk"),
        ("v", "Wv", "bv"),
        ("o", "Wo", "bo"),
    ):
        w_dr[nm] = nc.dram_tensor(w_name, [H, H], F32, kind="ExternalInput").ap()
        b_dr[nm] = nc.dram_tensor(b_name, [H], F32, kind="ExternalInput").ap()
    g_ln = nc.dram_tensor("ln_gamma", [H], F32, kind="ExternalInput").ap()
    beta_ln = nc.dram_tensor("ln_beta", [H], F32, kind="ExternalInput").ap()

    out_attn = nc.dram_tensor("attn_out", [S, H], F32, kind="ExternalOutput").ap()
    out_scores = nc.dram_tensor("scores", [NH, S, S], F32, kind="ExternalOutput").ap()
    out_probs = nc.dram_tensor("probs", [NH, S, S], F32, kind="ExternalOutput").ap()

    with tile.TileContext(nc) as tc:
        with tc.tile_pool(name="pers", bufs=1) as pers:
            # -------- persistent tensors (span several phases) --------
            qT = pers.tile([P, OC, S], BF16, tag="qT")  # q^T ints [o, t]
            kT = pers.tile([P, OC, S], BF16, tag="kT")
            v_int = pers.tile([P, TC, H], BF16, tag="v_int")  # v ints [t, o]
            a2 = pers.tile([P, OC, S], BF16, tag="a2")  # ctx_q^T ints [H, t]
            woT = pers.tile([P, OC, H], BF16, tag="woT")  # Wo'^T ints [i, o]
            ident = pers.tile([P, P], F32, tag="ident")
            make_identity(nc, ident[:])
            # per-o-channel bias carriers ([o%128, oc]) pre-scaled by 1/s
            b50 = {}
            for nm in ("q", "k"):
                braw = pers.tile([P, OC], F32, tag=f"braw_{nm}")
                nc.sync.dma_start(braw[:], b_dr[nm].rearrange("(oc p) -> p oc", p=P))
                b50[nm] = pers.tile([P, OC], F32, tag=f"b50_{nm}", name=f"b50_{nm}")
                nc.vector.tensor_scalar(b50[nm][:], braw[:], INV_S_ACT, None, MULT)
            bv_row = pers.tile([1, H], F32, tag="bv_row")
            nc.sync.dma_start(bv_row[:], b_dr["v"][None, :])
            bv50 = pers.tile([1, H], F32, tag="bv50")
            nc.vector.tensor_scalar(bv50[:], bv_row[:], INV_S_ACT, None, MULT)
            bv50_rep = pers.tile([P, H], F32, tag="bv50_rep")
            nc.gpsimd.partition_broadcast(bv50_rep[:], bv50[0:1, :])
            bo_row = pers.tile([1, H], F32, tag="bo_row")
            nc.sync.dma_start(bo_row[:], b_dr["o"][None, :])
            g_row = pers.tile([1, H], F32, tag="g_row")
            nc.sync.dma_start(g_row[:], g_ln[None, :])
            beta_row = pers.tile([1, H], F32, tag="beta_row")
            nc.sync.dma_start(beta_row[:], beta_ln[None, :])
            if mask_nonzero:
                mask_pp = pers.tile([P, TC], F32, tag="mask_pp")  # [k%128, kc]
                nc.sync.dma_start(mask_pp[:], msk.rearrange("(kc p) -> p kc", p=P))
                mask_row = pers.tile([1, S], F32, tag="mask_row")
                nc.sync.dma_start(mask_row[:], msk[None, :])
                maskc = pers.tile([1, S], F32, tag="maskc")  # mask / C_QK
                nc.vector.tensor_scalar(maskc[:], mask_row[:], 1.0 / C_QK, None, MULT)
                ones_row = pers.tile([1, S], F32, tag="ones_row")
                nc.vector.memset(ones_row[:], 1.0)
            # weight scales (s_w = absmax/127), one [P,1] carrier per weight
            w_scale = {
                nm: pers.tile([P, 1], F32, tag=f"ws_{nm}", name=f"ws_{nm}")
                for nm in w_dr
            }

            # -------- phase A/B: x^T, quantized weights, projections --
            with (
                tc.tile_pool(name="xT_pool", bufs=1) as xTp,
                tc.tile_pool(name="tr_psum", bufs=4, space="PSUM") as trp,
                tc.tile_pool(name="mm_psum", bufs=4, space="PSUM") as mmp,
                tc.tile_pool(name="epi", bufs=2) as epi,
                tc.tile_pool(name="wstage", bufs=1) as wsp,
                tc.tile_pool(name="wT_pool", bufs=1) as wTp,
                tc.tile_pool(name="stats", bufs=1) as stats,
            ):
                xT = xTp.tile([P, OC, S], F32R, tag="xT")  # x^T [i, t]
                x_sb = wsp.tile([P, TC, H], F32, tag="w_sb", name="x_sb")
                nc.sync.dma_start(x_sb[:], hid.rearrange("(t p) i -> p t i", p=P))
                _transpose_to(nc, trp, xT, x_sb, ident[:])

                for nm in ("o", "v", "q", "k"):
                    # ---- load + per-tensor quantize (in place) ----
                    w_sb = wsp.tile([P, OC, H], F32, tag="w_sb")
                    nc.sync.dma_start(
                        w_sb[:], w_dr[nm].rearrange("(oc p) i -> p oc i", p=P)
                    )
                    amax_p = stats.tile([P, 1], F32, tag=f"amax_p_{nm}")
                    nc.vector.tensor_reduce(
                        amax_p[:], w_sb[:], XY_AXIS, MAX, apply_absolute_value=True
                    )
                    amax = stats.tile([P, 1], F32, tag=f"amax_{nm}")
                    nc.gpsimd.partition_all_reduce(
                        amax[:], amax_p[:], channels=P, reduce_op=bass_isa.ReduceOp.max
                    )
                    nc.vector.tensor_scalar(
                        w_scale[nm][:], amax[:], 1.0 / 127.0, None, MULT
                    )
                    inv_s = stats.tile([P, 1], F32, tag=f"invs_{nm}")
                    nc.vector.reciprocal(inv_s[:], amax[:])
                    nc.vector.tensor_scalar(inv_s[:], inv_s[:], 127.0, None, MULT)
                    nc.vector.tensor_scalar(
                        w_sb[:], w_sb[:], inv_s[:], C_MAGIC, MULT, ADD
                    )
                    nc.vector.tensor_scalar(w_sb[:], w_sb[:], C_MAGIC, None, SUB)
                    # ---- transpose W' ints -> [i, o] ----
                    wT = wTp.tile([P, OC, H], F32R, tag="wT")
                    _transpose_to(nc, trp, wT, w_sb, ident[:])
                    if nm == "o":
                        nc.any.tensor_copy(out=woT[:], in_=wT[:])
                        continue
                    # swi = s_w / s_act  (fold weight scale + act quant step)
                    swi = stats.tile([P, 1], F32, tag=f"swi_{nm}")
                    nc.vector.tensor_scalar(
                        swi[:], w_scale[nm][:], INV_S_ACT, None, MULT
                    )
                    if nm in ("q", "k"):
                        out_int = qT if nm == "q" else kT
                        # GEMM -> [o_p, t_f]
                        for oc in range(OC):
                            for th in range(2):
                                ps = mmp.tile([P, 512], F32, tag="proj_ps")
                                for ic in range(OC):
                                    nc.tensor.matmul(
                                        ps[:],
                                        wT[:, ic, oc * P : (oc + 1) * P],
                                        xT[:, ic, th * 512 : (th + 1) * 512],
                                        start=(ic == 0),
                                        stop=(ic == OC - 1),
                                    )
                                t1 = epi.tile([P, 512], F32, tag="prj_t1")
                                nc.vector.tensor_scalar(
                                    t1[:],
                                    ps[:],
                                    swi[:],
                                    b50[nm][:, oc : oc + 1],
                                    MULT,
                                    ADD,
                                )
                                nc.vector.tensor_scalar(
                                    t1[:], t1[:], 127.0, -127.0, MIN, MAX
                                )
                                nc.vector.tensor_scalar(
                                    out_int[:, oc, th * 512 : (th + 1) * 512],
                                    t1[:],
                                    C_MAGIC,
                                    C_MAGIC,
                                    ADD,
                                    SUB,
                                )
                    else:  # v: GEMM -> [t_p, o_f]
                        for tcc in range(TC):
                            for oh in range(2):
                                ps = mmp.tile([P, 512], F32, tag="proj_ps")
                                for ic in range(OC):
                                    nc.tensor.matmul(
                                        ps[:],
                                        xT[:, ic, tcc * P : (tcc + 1) * P],
                                        wT[:, ic, oh * 512 : (oh + 1) * 512],
                                        start=(ic == 0),
                                        stop=(ic == OC - 1),
                                    )
                                t1 = epi.tile([P, 512], F32, tag="prj_t1")
                                nc.vector.scalar_tensor_tensor(
                                    t1[:],
                                    ps[:],
                                    swi[:],
                                    bv50_rep[:, oh * 512 : (oh + 1) * 512],
                                    MULT,
                                    ADD,
                                )
                                nc.vector.tensor_scalar(
                                    t1[:], t1[:], 127.0, -127.0, MIN, MAX
                                )
                                nc.vector.tensor_scalar(
                                    v_int[:, tcc, oh * 512 : (oh + 1) * 512],
                                    t1[:],
                                    C_MAGIC,
                                    C_MAGIC,
                                    ADD,
                                    SUB,
                                )

            # -------- per-head attention ------------------------------
            with (
                tc.tile_pool(name="rr_dram", bufs=2, space="DRAM") as rrdp,
                tc.tile_pool(name="l1_psum", bufs=2, space="PSUM") as p1pool,
                tc.tile_pool(name="l2_psum", bufs=1, space="PSUM") as p2pool,
                tc.tile_pool(name="ctx_psum", bufs=1, space="PSUM") as pcpool,
                tc.tile_pool(name="l1", bufs=3) as l1pool,
                tc.tile_pool(name="l2", bufs=3) as l2pool,
                tc.tile_pool(name="hstat", bufs=3) as hstat,
                tc.tile_pool(name="ctxepi", bufs=2) as ctxepi,
            ):
                ctx_tiles = {}
                head_state = {}

                def emit_l1(h):
                    hp = h % 2
                    oc_h = h // 2
                    plo = D * hp
                    qT_h = qT[plo : plo + D, oc_h, :]  # [64, 1024] ints
                    kT_h = kT[plo : plo + D, oc_h, :]

                    rs = hstat.tile([P, TC], F32, tag="rs", name="rs")
                    rr = hstat.tile([P, TC], F32, tag="rr", name="rr")
                    rr50 = hstat.tile([P, TC], F32, tag="rr50", name="rr50")
                    rrT = hstat.tile([1, S], F32, tag="rrT", name="rrT")

                    # ---- layout 1: scores [q, k] ----
                    for qc in range(TC):
                        ps1 = p1pool.tile([P, S], F32, tag="ps1")
                        for kh in range(2):
                            nc.tensor.matmul(
                                ps1[:, kh * 512 : (kh + 1) * 512],
                                qT_h[:, qc * P : (qc + 1) * P],
                                kT_h[:, kh * 512 : (kh + 1) * 512],
                                start=True,
                                stop=not mask_nonzero,
                            )
                            if mask_nonzero:
                                nc.tensor.matmul(
                                    ps1[:, kh * 512 : (kh + 1) * 512],
                                    ones_row[0:1, qc * P : (qc + 1) * P],
                                    maskc[0:1, kh * 512 : (kh + 1) * 512],
                                    start=False,
                                    stop=True,
                                    skip_group_check=True,
                                )
                        sc_t = l1pool.tile([P, S], F32, tag="sc")
                        nc.scalar.activation(sc_t[:], ps1[:], Copy, scale=C_QK)
                        nc.sync.dma_start(
                            out_scores[h, qc * P : (qc + 1) * P, :], sc_t[:]
                        )
                        un_t = l1pool.tile([P, S], F32, tag="un")
                        nc.scalar.activation(
                            un_t[:],
                            ps1[:],
                            Exp,
                            scale=C_QK,
                            accum_out=rs[:, qc : qc + 1],
                        )
                        nc.vector.reciprocal(rr[:, qc : qc + 1], rs[:, qc : qc + 1])
                        pr_t = l1pool.tile([P, S], F32, tag="pr")
                        nc.vector.tensor_scalar(
                            pr_t[:], un_t[:], rr[:, qc : qc + 1], None, MULT
                        )
                        nc.sync.dma_start(
                            out_probs[h, qc * P : (qc + 1) * P, :], pr_t[:]
                        )

                    nc.vector.tensor_scalar(rr50[:], rr[:], INV_S_ACT, None, MULT)
                    # scatter [128, TC] -> flat [S] (t = qc*128 + p) via DRAM
                    rr_d = rrdp.tile([S], F32, tag="rr_d")
                    with nc.allow_non_contiguous_dma(reason="tiny per-head stats"):
                        nc.sync.dma_start(
                            rr_d[:].rearrange("(c p) -> p c", p=P), rr50[:, :]
                        )
                    nc.sync.dma_start(rrT[:], rr_d[None, :])
                    rrT_rep = hstat.tile([P, S], F32, tag="rrT_rep", name="rrT_rep")
                    nc.gpsimd.partition_broadcast(rrT_rep[:], rrT[0:1, :])
                    head_state[h] = rrT_rep

                def emit_l2(h):
                    hp = h % 2
                    oc_h = h // 2
                    plo = D * hp
                    qT_h = qT[plo : plo + D, oc_h, :]
                    kT_h = kT[plo : plo + D, oc_h, :]
                    rrT_rep = head_state.pop(h)

                    # ---- layout 2: scores^T [k, q] feeding PV ----
                    if hp == 0:
                        ctx_tiles[oc_h] = pcpool.tile(
                            [P, S], F32, tag="ctx_ps", name="ctx_ps"
                        )
                    ctx_ps = ctx_tiles[oc_h]
                    for kc in range(TC):
                        ps2 = p2pool.tile([P, S], F32, tag="ps2")
                        for qh in range(2):
                            nc.tensor.matmul(
                                ps2[:, qh * 512 : (qh + 1) * 512],
                                kT_h[:, kc * P : (kc + 1) * P],
                                qT_h[:, qh * 512 : (qh + 1) * 512],
                                start=True,
                                stop=True,
                            )
                        un2_t = l2pool.tile([P, S], F32, tag="un2")
                        if mask_nonzero:
                            nc.scalar.activation(
                                un2_t[:],
                                ps2[:],
                                Exp,
                                scale=C_QK,
                                bias=mask_pp[:, kc : kc + 1],
                            )
                        else:
                            nc.scalar.activation(un2_t[:], ps2[:], Exp, scale=C_QK)
                        t2_t = l2pool.tile([P, S], F32, tag="t2")
                        nc.gpsimd.tensor_tensor(t2_t[:], un2_t[:], rrT_rep[:], MULT)
                        aT_t = l2pool.tile([P, S], BF16, tag="aT")
                        nc.vector.tensor_scalar(
                            aT_t[:], t2_t[:], C_MAGIC, C_MAGIC, ADD, SUB
                        )
                        for qh in range(2):
                            nc.tensor.matmul(
                                ctx_ps[plo : plo + D, qh * 512 : (qh + 1) * 512],
                                v_int[:, kc, h * D : (h + 1) * D],
                                aT_t[:, qh * 512 : (qh + 1) * 512],
                                start=(kc == 0),
                                stop=(kc == TC - 1),
                                skip_group_check=True,
                            )

                    if hp == 1:
                        # fake-quant ctx^T pair -> a2[:, oc_h, :] (ints, bf16)
                        ctx_tiles.pop(oc_h)
                        c_pv = float(np.float32(S_ACT * S_ACT) * np.float32(INV_S_ACT))
                        e1 = ctxepi.tile([P, S], F32, tag="ce1", name="e1")
                        nc.vector.tensor_scalar(
                            e1[:], ctx_ps[:], c_pv, 127.0, MULT, MIN
                        )
                        nc.vector.tensor_scalar(
                            e1[:], e1[:], -127.0, C_MAGIC, MAX, ADD
                        )
                        nc.vector.tensor_scalar(
                            a2[:, oc_h, :], e1[:], C_MAGIC, None, SUB
                        )

                # software pipeline: L1(h) overlaps L2(h-1) in every
                # engine's program order
                emit_l1(0)
                for h in range(1, NH):
                    emit_l1(h)
                    emit_l2(h - 1)
                emit_l2(NH - 1)

            # -------- output projection + residual + LayerNorm --------
            with (
                tc.tile_pool(name="o_psum", bufs=2, space="PSUM") as opool,
                tc.tile_pool(name="fin", bufs=3) as fin,
                tc.tile_pool(name="rows", bufs=1) as rows,
                tc.tile_pool(name="lnstat", bufs=2) as lnstat,
            ):
                bo_rep = rows.tile([P, H], F32, tag="bo_rep")
                nc.gpsimd.partition_broadcast(bo_rep[:], bo_row[0:1, :])
                g_rep = rows.tile([P, H], F32, tag="g_rep")
                nc.gpsimd.partition_broadcast(g_rep[:], g_row[0:1, :])
                beta_rep = rows.tile([P, H], F32, tag="beta_rep")
                nc.gpsimd.partition_broadcast(beta_rep[:], beta_row[0:1, :])
                c_o = lnstat.tile([P, 1], F32, tag="c_o")  # s_ctx * s_wo
                nc.vector.tensor_scalar(
                    c_o[:], w_scale["o"][:], float(S_ACT), None, MULT
                )
                eps_t = lnstat.tile([P, 1], F32, tag="eps_t")
                nc.vector.memset(eps_t[:], LN_EPS)
                for tcc in range(TC):
                    ps = opool.tile([P, H], F32, tag="ops")
                    for oh in range(2):
                        for hc in range(OC):
                            nc.tensor.matmul(
                                ps[:, oh * 512 : (oh + 1) * 512],
                                a2[:, hc, tcc * P : (tcc + 1) * P],
                                woT[:, hc, oh * 512 : (oh + 1) * 512],
                                start=(hc == 0),
                                stop=(hc == OC - 1),
                            )
                    hid_t = fin.tile([P, H], F32, tag="hid_t")
                    nc.sync.dma_start(hid_t[:], hid[tcc * P : (tcc + 1) * P, :])
                    nc.vector.tensor_tensor(hid_t[:], hid_t[:], bo_rep[:], ADD)
                    x_t = fin.tile([P, H], F32, tag="x_t")
                    nc.vector.scalar_tensor_tensor(
                        x_t[:], ps[:], c_o[:], hid_t[:], MULT, ADD
                    )
                    # LayerNorm over the free (H) dim
                    xsum = lnstat.tile([P, 1], F32, tag="xsum")
                    nc.vector.tensor_reduce(xsum[:], x_t[:], XY_AXIS, ADD)
                    mu = lnstat.tile([P, 1], F32, tag="mu")
                    nc.vector.tensor_scalar(mu[:], xsum[:], 1.0 / H, None, MULT)
                    xc_t = fin.tile([P, H], F32, tag="xc_t")
                    varsum = lnstat.tile([P, 1], F32, tag="varsum")
                    nc.vector.tensor_scalar(xc_t[:], x_t[:], mu[:], None, SUB)
                    # square into the spent hid_t slot; only accum_out matters
                    nc.vector.scalar_tensor_tensor(
                        hid_t[:], xc_t[:], 1.0, xc_t[:], MULT, MULT,
                        accum_out=varsum[:],
                    )
                    std = lnstat.tile([P, 1], F32, tag="std")
                    nc.scalar.activation(
                        std[:], varsum[:], Sqrt, scale=1.0 / H, bias=eps_t[:]
                    )
                    rstd = lnstat.tile([P, 1], F32, tag="rstd")
                    nc.vector.reciprocal(rstd[:], std[:])
                    nc.vector.tensor_scalar(xc_t[:], xc_t[:], rstd[:], None, MULT)
                    nc.vector.tensor_tensor(xc_t[:], xc_t[:], g_rep[:], MULT)
                    nc.vector.tensor_tensor(xc_t[:], xc_t[:], beta_rep[:], ADD)
                    nc.sync.dma_start(out_attn[tcc * P : (tcc + 1) * P, :], xc_t[:])

    nc.compile()
    return nc


_CACHE = {}


def _get_nc(mask_nonzero):
    key = bool(mask_nonzero)
    if key not in _CACHE:
        _CACHE[key] = build(key)
    return _CACHE[key]


def kernel(
    hidden_states,
    attention_mask,
    Wq,
    bq,
    Wk,
    bk,
    Wv,
    bv,
    Wo,
    bo,
    ln_gamma,
    ln_beta,
    trace=False,
    **trace_kwargs,
):
    hidden_states = np.asarray(hidden_states, dtype=np.float32)
    attention_mask = np.asarray(attention_mask, dtype=np.float32)
    B = hidden_states.shape[0]
    assert B == 8 and hidden_states.shape[1:] == (S, H)

    mask_nonzero = bool(np.any(attention_mask != 0.0))
    nc = _get_nc(mask_nonzero)

    shared = {
        "Wq": np.ascontiguousarray(Wq, np.float32),
        "Wk": np.ascontiguousarray(Wk, np.float32),
        "Wv": np.ascontiguousarray(Wv, np.float32),
        "Wo": np.ascontiguousarray(Wo, np.float32),
        "bq": np.ascontiguousarray(bq, np.float32),
        "bk": np.ascontiguousarray(bk, np.float32),
        "bv": np.ascontiguousarray(bv, np.float32),
        "bo": np.ascontiguousarray(bo, np.float32),
        "ln_gamma": np.ascontiguousarray(ln_gamma, np.float32),
        "ln_beta": np.ascontiguousarray(ln_beta, np.float32),
    }
    in_maps = []
    for b in range(B):
        m = dict(shared)
        m["hidden"] = np.ascontiguousarray(hidden_states[b])
        m["mask"] = np.ascontiguousarray(
            np.broadcast_to(attention_mask[b], (1, 1, S)).reshape(S)
        )
        in_maps.append(m)

    res = run_bass_kernel_spmd(
        nc, in_maps, core_ids=list(range(8)), trace=trace, **trace_kwargs
    )
    attn = np.stack([res.results[b]["attn_out"] for b in range(B)])
    scores = np.stack([res.results[b]["scores"] for b in range(B)])
    probs = np.stack([res.results[b]["probs"] for b in range(B)])
    kernel.last_results = res
    return attn, scores, probs


# revision 15
# speedup vs baseline: 2.8942x; 1.0761x over previous
"""BertAttention (quantized, eval) Trainium2 kernel.

Data-parallel over batch: 8 batch elements -> 8 NeuronCores, one full
attention layer per core, no collectives.  Returns the same tuple as the
reference: (attention_output, scores, probs).

Key ideas:
- Every fake-quantized tensor (weights, q, k, v, probs, ctx) is
  round(x/s) * s with round(x/s) an integer in [-127, 127].  We carry the
  integers (exact in bf16) through the TensorEngine, so QK^T, PV and the
  output projection are exact integer matmuls accumulated in f32 PSUM
  (max |sum| < 2^24); the f32 scales are applied in the epilogues.
- round-half-to-even == (x + 1.5*2^23) - 1.5*2^23 in f32, one 2-op
  tensor_scalar instruction (matches jnp.round for |x| <= 2^22).
- scores are computed in both [q,k] layout (scores/probs outputs, softmax
  row sums via the ACT engine's accum_out) and [k,q] layout (so the PV
  matmul gets probs^T without transposing the 64MB probs tensor).  Both
  layouts reuse the same Q^T / K^T integer operands.
"""

import os
import sys

sys.path.insert(0, "/opt/trn_rl_repo")
# 256B DRAM pages shatter DMA descriptors (2.5KB packets, ~225GB/s);
# 4KB pages let 4KB rows move as whole descriptors.
os.environ.setdefault("NEURON_SCRATCHPAD_PAGE_SIZE", "4096")

import numpy as np

import concourse.bass as bass
import concourse.bass_isa as bass_isa
import concourse.tile as tile
from concourse import bacc, mybir
from concourse.bass_utils import run_bass_kernel_spmd
from concourse.masks import make_identity

H = 1024
S = 1024
NH = 16
D = 64
P = 128
OC = H // P  # 8 chunks of 128 output channels
TC = S // P  # 8 chunks of 128 tokens
F32 = mybir.dt.float32
F32R = mybir.dt.float32r
BF16 = mybir.dt.bfloat16

CLIP = np.float32(2.5)
N_LEVELS = np.float32(127.0)
S_ACT = np.float32(CLIP / N_LEVELS)  # activation quant step
INV_S_ACT = float(np.float32(1.0) / S_ACT)  # ~50.8
C_QK = float(np.float32(S_ACT * S_ACT) / np.float32(8.0))  # sqrt(D) == 8
C_MAGIC = 12582912.0  # 1.5 * 2**23: f32 round-to-nearest-even bias
LN_EPS = 1e-12

ADD = mybir.AluOpType.add
SUB = mybir.AluOpType.subtract
MULT = mybir.AluOpType.mult
MIN = mybir.AluOpType.min
MAX = mybir.AluOpType.max
BYPASS = mybir.AluOpType.bypass
Exp = mybir.ActivationFunctionType.Exp
Copy = mybir.ActivationFunctionType.Copy
Sqrt = mybir.ActivationFunctionType.Sqrt
XY_AXIS = mybir.AxisListType.XY


def _transpose_to(nc, psum_pool, dst, src, ident):
    """PE-transpose [P, C, J*P] -> [P, J, C*P]: each 128x128 tile
    src[:, c, j*P:+P] lands transposed in dst[:, j, c*P:+P]."""
    n_c = src.shape[1]
    n_j = src.shape[2] // P
    for c in range(n_c):
        for j in range(n_j):
            pt = psum_pool.tile([P, P], F32, tag="tr_psum")
            nc.tensor.transpose(pt[:], src[:, c, j * P : (j + 1) * P], ident)
            nc.any.tensor_copy(out=dst[:, j, c * P : (c + 1) * P], in_=pt[:])


def build(mask_nonzero: bool):
    nc = bacc.Bacc("TRN2", target_bir_lowering=False, debug=False, num_devices=8)

    hid = nc.dram_tensor("hidden", [S, H], F32, kind="ExternalInput").ap()
    msk = nc.dram_tensor("mask", [S], F32, kind="ExternalInput").ap()
    w_dr = {}
    b_dr = {}
    for nm, w_name, b_name in (
        ("q", "Wq", "bq"),
        ("k", "Wk", "bk"),
        ("v", "Wv", "bv"),
        ("o", "Wo", "bo"),
    ):
        w_dr[nm] = nc.dram_tensor(w_name, [H, H], F32, kind="ExternalInput").ap()
        b_dr[nm] = nc.dram_tensor(b_name, [H], F32, kind="ExternalInput").ap()
    g_ln = nc.dram_tensor("ln_gamma", [H], F32, kind="ExternalInput").ap()
    beta_ln = nc.dram_tensor("ln_beta", [H], F32, kind="ExternalInput").ap()

    out_attn = nc.dram_tensor("attn_out", [S, H], F32, kind="ExternalOutput").ap()
    out_scores = nc.dram_tensor("scores", [NH, S, S], F32, kind="ExternalOutput").ap()
    out_probs = nc.dram_tensor("probs", [NH, S, S], F32, kind="ExternalOutput").ap()

    with tile.TileContext(nc) as tc:
        with tc.tile_pool(name="pers", bufs=1) as pers:
            # -------- persistent tensors (span several phases) --------
            qT = pers.tile([P, OC, S], BF16, tag="qT")  # q^T ints [o, t]
            kT = pers.tile([P, OC, S], BF16, tag="kT")
            v_int = pers.tile([P, TC, H], BF16, tag="v_int")  # v ints [t, o]
            a2 = pers.tile([P, OC, S], BF16, tag="a2")  # ctx_q^T ints [H, t]
            woT = pers.tile([P, OC, H], BF16, tag="woT")  # Wo'^T ints [i, o]
            ident = pers.tile([P, P], F32, tag="ident")
            make_identity(nc, ident[:])
            # per-o-channel bias carriers ([o%128, oc]) pre-scaled by 1/s
            b50 = {}
            for nm in ("q", "k"):
                braw = pers.tile([P, OC], F32, tag=f"braw_{nm}")
                nc.sync.dma_start(braw[:], b_dr[nm].rearrange("(oc p) -> p oc", p=P))
                b50[nm] = pers.tile([P, OC], F32, tag=f"b50_{nm}", name=f"b50_{nm}")
                nc.vector.tensor_scalar(b50[nm][:], braw[:], INV_S_ACT, None, MULT)
            bv_row = pers.tile([1, H], F32, tag="bv_row")
            nc.sync.dma_start(bv_row[:], b_dr["v"][None, :])
            bv50 = pers.tile([1, H], F32, tag="bv50")
            nc.vector.tensor_scalar(bv50[:], bv_row[:], INV_S_ACT, None, MULT)
            bv50_rep = pers.tile([P, H], F32, tag="bv50_rep")
            nc.gpsimd.partition_broadcast(bv50_rep[:], bv50[0:1, :])
            bo_row = pers.tile([1, H], F32, tag="bo_row")
            nc.sync.dma_start(bo_row[:], b_dr["o"][None, :])
            g_row = pers.tile([1, H], F32, tag="g_row")
            nc.sync.dma_start(g_row[:], g_ln[None, :])
            beta_row = pers.tile([1, H], F32, tag="beta_row")
            nc.sync.dma_start(beta_row[:], beta_ln[None, :])
            if mask_nonzero:
                mask_pp = pers.tile([P, TC], F32, tag="mask_pp")  # [k%128, kc]
                nc.sync.dma_start(mask_pp[:], msk.rearrange("(kc p) -> p kc", p=P))
                mask_row = pers.tile([1, S], F32, tag="mask_row")
                nc.sync.dma_start(mask_row[:], msk[None, :])
                maskc = pers.tile([1, S], F32, tag="maskc")  # mask / C_QK
                nc.vector.tensor_scalar(maskc[:], mask_row[:], 1.0 / C_QK, None, MULT)
                ones_row = pers.tile([1, S], F32, tag="ones_row")
                nc.vector.memset(ones_row[:], 1.0)
            # weight scales (s_w = absmax/127), one [P,1] carrier per weight
            w_scale = {
                nm: pers.tile([P, 1], F32, tag=f"ws_{nm}", name=f"ws_{nm}")
                for nm in w_dr
            }

            # -------- phase A/B: x^T, quantized weights, projections --
            with (
                tc.tile_pool(name="xT_pool", bufs=1) as xTp,
                tc.tile_pool(name="tr_psum", bufs=4, space="PSUM") as trp,
                tc.tile_pool(name="mm_psum", bufs=4, space="PSUM") as mmp,
                tc.tile_pool(name="epi", bufs=2) as epi,
                tc.tile_pool(name="wstage", bufs=1) as wsp,
                tc.tile_pool(name="wT_pool", bufs=1) as wTp,
                tc.tile_pool(name="stats", bufs=1) as stats,
            ):
                xT = xTp.tile([P, OC, S], F32R, tag="xT")  # x^T [i, t]
                x_sb = wsp.tile([P, TC, H], F32, tag="w_sb", name="x_sb")
                nc.sync.dma_start(x_sb[:], hid.rearrange("(t p) i -> p t i", p=P))
                _transpose_to(nc, trp, xT, x_sb, ident[:])

                for nm in ("o", "v", "q", "k"):
                    # ---- load + per-tensor quantize (in place) ----
                    w_sb = wsp.tile([P, OC, H], F32, tag="w_sb")
                    nc.sync.dma_start(
                        w_sb[:], w_dr[nm].rearrange("(oc p) i -> p oc i", p=P)
                    )
                    amax_p = stats.tile([P, 1], F32, tag=f"amax_p_{nm}")
                    nc.vector.tensor_reduce(
                        amax_p[:], w_sb[:], XY_AXIS, MAX, apply_absolute_value=True
                    )
                    amax = stats.tile([P, 1], F32, tag=f"amax_{nm}")
                    nc.gpsimd.partition_all_reduce(
                        amax[:], amax_p[:], channels=P, reduce_op=bass_isa.ReduceOp.max
                    )
                    nc.vector.tensor_scalar(
                        w_scale[nm][:], amax[:], 1.0 / 127.0, None, MULT
                    )
                    inv_s = stats.tile([P, 1], F32, tag=f"invs_{nm}")
                    nc.vector.reciprocal(inv_s[:], amax[:])
                    nc.vector.tensor_scalar(inv_s[:], inv_s[:], 127.0, None, MULT)
                    nc.vector.tensor_scalar(
                        w_sb[:], w_sb[:], inv_s[:], C_MAGIC, MULT, ADD
                    )
                    nc.vector.tensor_scalar(w_sb[:], w_sb[:], C_MAGIC, None, SUB)
                    # ---- transpose W' ints -> [i, o] ----
                    wT = wTp.tile([P, OC, H], F32R, tag="wT")
                    _transpose_to(nc, trp, wT, w_sb, ident[:])
                    if nm == "o":
                        nc.any.tensor_copy(out=woT[:], in_=wT[:])
                        continue
                    # swi = s_w / s_act  (fold weight scale + act quant step)
                    swi = stats.tile([P, 1], F32, tag=f"swi_{nm}")
                    nc.vector.tensor_scalar(
                        swi[:], w_scale[nm][:], INV_S_ACT, None, MULT
                    )
                    if nm in ("q", "k"):
                        out_int = qT if nm == "q" else kT
                        # GEMM -> [o_p, t_f]
                        for oc in range(OC):
                            for th in range(2):
                                ps = mmp.tile([P, 512], F32, tag="proj_ps")
                                for ic in range(OC):
                                    nc.tensor.matmul(
                                        ps[:],
                                        wT[:, ic, oc * P : (oc + 1) * P],
                                        xT[:, ic, th * 512 : (th + 1) * 512],
                                        start=(ic == 0),
                                        stop=(ic == OC - 1),
                                    )
                                t1 = epi.tile([P, 512], F32, tag="prj_t1")
                                nc.vector.tensor_scalar(
                                    t1[:],
                                    ps[:],
                                    swi[:],
                                    b50[nm][:, oc : oc + 1],
                                    MULT,
                                    ADD,
                                )
                                nc.vector.tensor_scalar(
                                    t1[:], t1[:], 127.0, -127.0, MIN, MAX
                                )
                                nc.vector.tensor_scalar(
                                    out_int[:, oc, th * 512 : (th + 1) * 512],
                                    t1[:],
                                    C_MAGIC,
                                    C_MAGIC,
                                    ADD,
                                    SUB,
                                )
                    else:  # v: GEMM -> [t_p, o_f]
                        for tcc in range(TC):
                            for oh in range(2):
                                ps = mmp.tile([P, 512], F32, tag="proj_ps")
                                for ic in range(OC):
                                    nc.tensor.matmul(
                                        ps[:],
                                        xT[:, ic, tcc * P : (tcc + 1) * P],
                                        wT[:, ic, oh * 512 : (oh + 1) * 512],
                                        start=(ic == 0),
                                        stop=(ic == OC - 1),
                                    )
                                t1 = epi.tile([P, 512], F32, tag="prj_t1")
                                nc.vector.scalar_tensor_tensor(
                                    t1[:],
                                    ps[:],
                                    swi[:],
                                    bv50_rep[:, oh * 512 : (oh + 1) * 512],
                                    MULT,
                                    ADD,
                                )
                                nc.vector.tensor_scalar(
                                    t1[:], t1[:], 127.0, -127.0, MIN, MAX
                                )
                                nc.vector.tensor_scalar(
                                    v_int[:, tcc, oh * 512 : (oh + 1) * 512],
                                    t1[:],
                                    C_MAGIC,
                                    C_MAGIC,
                                    ADD,
                                    SUB,
                                )

            # -------- per-head attention ------------------------------
            with (
                tc.tile_pool(name="rr_dram", bufs=2, space="DRAM") as rrdp,
                tc.tile_pool(name="l1_psum", bufs=2, space="PSUM") as p1pool,
                tc.tile_pool(name="l2_psum", bufs=1, space="PSUM") as p2pool,
                tc.tile_pool(name="ctx_psum", bufs=1, space="PSUM") as pcpool,
                tc.tile_pool(name="l1", bufs=3) as l1pool,
                tc.tile_pool(name="l2", bufs=3) as l2pool,
                tc.tile_pool(name="hstat", bufs=3) as hstat,
                tc.tile_pool(name="ctxepi", bufs=2) as ctxepi,
            ):
                ctx_tiles = {}
                head_state = {}

                def l1_gen(h):
                    hp = h % 2
                    oc_h = h // 2
                    plo = D * hp
                    qT_h = qT[plo : plo + D, oc_h, :]  # [64, 1024] ints
                    kT_h = kT[plo : plo + D, oc_h, :]

                    rs = hstat.tile([P, TC], F32, tag="rs", name="rs")
                    rr = hstat.tile([P, TC], F32, tag="rr", name="rr")
                    rr50 = hstat.tile([P, TC], F32, tag="rr50", name="rr50")
                    rrT = hstat.tile([1, S], F32, tag="rrT", name="rrT")

                    # ---- layout 1: scores [q, k] ----
                    for qc in range(TC):
                        yield
                        ps1 = p1pool.tile([P, S], F32, tag="ps1", name="ps1")
                        for kh in range(2):
                            nc.tensor.matmul(
                                ps1[:, kh * 512 : (kh + 1) * 512],
                                qT_h[:, qc * P : (qc + 1) * P],
                                kT_h[:, kh * 512 : (kh + 1) * 512],
                                start=True,
                                stop=not mask_nonzero,
                            )
                            if mask_nonzero:
                                nc.tensor.matmul(
                                    ps1[:, kh * 512 : (kh + 1) * 512],
                                    ones_row[0:1, qc * P : (qc + 1) * P],
                                    maskc[0:1, kh * 512 : (kh + 1) * 512],
                                    start=False,
                                    stop=True,
                                    skip_group_check=True,
                                )
                        sc_t = l1pool.tile([P, S], F32, tag="sc")
                        nc.scalar.activation(sc_t[:], ps1[:], Copy, scale=C_QK)
                        nc.sync.dma_start(
                            out_scores[h, qc * P : (qc + 1) * P, :], sc_t[:]
                        )
                        un_t = l1pool.tile([P, S], F32, tag="un")
                        nc.scalar.activation(
                            un_t[:],
                            ps1[:],
                            Exp,
                            scale=C_QK,
                            accum_out=rs[:, qc : qc + 1],
                        )
                        nc.vector.reciprocal(rr[:, qc : qc + 1], rs[:, qc : qc + 1])
                        pr_t = l1pool.tile([P, S], F32, tag="pr")
                        nc.vector.tensor_scalar(
                            pr_t[:], un_t[:], rr[:, qc : qc + 1], None, MULT
                        )
                        nc.sync.dma_start(
                            out_probs[h, qc * P : (qc + 1) * P, :], pr_t[:]
                        )

                    nc.vector.tensor_scalar(rr50[:], rr[:], INV_S_ACT, None, MULT)
                    # scatter [128, TC] -> flat [S] (t = qc*128 + p) via DRAM
                    rr_d = rrdp.tile([S], F32, tag="rr_d")
                    with nc.allow_non_contiguous_dma(reason="tiny per-head stats"):
                        nc.sync.dma_start(
                            rr_d[:].rearrange("(c p) -> p c", p=P), rr50[:, :]
                        )
                    nc.sync.dma_start(rrT[:], rr_d[None, :])
                    rrT_rep = hstat.tile([P, S], F32, tag="rrT_rep", name="rrT_rep")
                    nc.gpsimd.partition_broadcast(rrT_rep[:], rrT[0:1, :])
                    head_state[h] = rrT_rep

                def l2_gen(h):
                    hp = h % 2
                    oc_h = h // 2
                    plo = D * hp
                    qT_h = qT[plo : plo + D, oc_h, :]
                    kT_h = kT[plo : plo + D, oc_h, :]
                    rrT_rep = head_state.pop(h)

                    # ---- layout 2: scores^T [k, q] feeding PV ----
                    if hp == 0:
                        ctx_tiles[oc_h] = pcpool.tile(
                            [P, S], F32, tag="ctx_ps", name="ctx_ps"
                        )
                    ctx_ps = ctx_tiles[oc_h]
                    for kc in range(TC):
                        yield
                        ps2 = p2pool.tile([P, S], F32, tag="ps2", name="ps2")
                        for qh in range(2):
                            nc.tensor.matmul(
                                ps2[:, qh * 512 : (qh + 1) * 512],
                                kT_h[:, kc * P : (kc + 1) * P],
                                qT_h[:, qh * 512 : (qh + 1) * 512],
                                start=True,
                                stop=True,
                            )
                        un2_t = l2pool.tile([P, S], F32, tag="un2", name="un2")
                        if mask_nonzero:
                            nc.scalar.activation(
                                un2_t[:],
                                ps2[:],
                                Exp,
                                scale=C_QK,
                                bias=mask_pp[:, kc : kc + 1],
                            )
                        else:
                            nc.scalar.activation(un2_t[:], ps2[:], Exp, scale=C_QK)
                        t2_t = l2pool.tile([P, S], F32, tag="t2", name="t2")
                        aT_t = l2pool.tile([P, S], BF16, tag="aT", name="aT")
                        for qh in range(2):
                            sl = slice(qh * 512, (qh + 1) * 512)
                            eng = nc.gpsimd if qh == 0 else nc.vector
                            eng.tensor_tensor(
                                t2_t[:, sl], un2_t[:, sl], rrT_rep[:, sl], MULT
                            )
                            nc.vector.tensor_scalar(
                                aT_t[:, sl], t2_t[:, sl], C_MAGIC, C_MAGIC, ADD, SUB
                            )
                            nc.tensor.matmul(
                                ctx_ps[plo : plo + D, sl],
                                v_int[:, kc, h * D : (h + 1) * D],
                                aT_t[:, sl],
                                start=(kc == 0),
                                stop=(kc == TC - 1),
                                skip_group_check=True,
                            )

                    if hp == 1:
                        # fake-quant ctx^T pair -> a2[:, oc_h, :] (ints, bf16)
                        ctx_tiles.pop(oc_h)
                        c_pv = float(np.float32(S_ACT * S_ACT) * np.float32(INV_S_ACT))
                        e1 = ctxepi.tile([P, S], F32, tag="ce1", name="e1")
                        nc.vector.tensor_scalar(
                            e1[:], ctx_ps[:], c_pv, 127.0, MULT, MIN
                        )
                        nc.vector.tensor_scalar(
                            e1[:], e1[:], -127.0, C_MAGIC, MAX, ADD
                        )
                        nc.vector.tensor_scalar(
                            a2[:, oc_h, :], e1[:], C_MAGIC, None, SUB
                        )

                # software pipeline: interleave L1(h) and L2(h-1) at tile
                # granularity so no engine's in-order stream stalls long
                def drive(g1, g2):
                    while True:
                        done1 = done2 = False
                        try:
                            next(g1)
                        except StopIteration:
                            done1 = True
                        try:
                            next(g2)
                        except StopIteration:
                            done2 = True
                        if done1 and done2:
                            return

                def empty_gen():
                    return iter(())

                prev = empty_gen()
                for h in range(NH):
                    drive(l1_gen(h), prev)
                    prev = l2_gen(h)
                drive(empty_gen(), prev)

            # -------- output projection + residual + LayerNorm --------
            with (
                tc.tile_pool(name="o_psum", bufs=2, space="PSUM") as opool,
                tc.tile_pool(name="fin", bufs=3) as fin,
                tc.tile_pool(name="rows", bufs=1) as rows,
                tc.tile_pool(name="lnstat", bufs=2) as lnstat,
            ):
                bo_rep = rows.tile([P, H], F32, tag="bo_rep")
                nc.gpsimd.partition_broadcast(bo_rep[:], bo_row[0:1, :])
                g_rep = rows.tile([P, H], F32, tag="g_rep")
                nc.gpsimd.partition_broadcast(g_rep[:], g_row[0:1, :])
                beta_rep = rows.tile([P, H], F32, tag="beta_rep")
                nc.gpsimd.partition_broadcast(beta_rep[:], beta_row[0:1, :])
                c_o = lnstat.tile([P, 1], F32, tag="c_o")  # s_ctx * s_wo
                nc.vector.tensor_scalar(
                    c_o[:], w_scale["o"][:], float(S_ACT), None, MULT
                )
                eps_t = lnstat.tile([P, 1], F32, tag="eps_t")
                nc.vector.memset(eps_t[:], LN_EPS)
                for tcc in range(TC):
                    ps = opool.tile([P, H], F32, tag="ops")
                    for oh in range(2):
                        for hc in range(OC):
                            nc.tensor.matmul(
                                ps[:, oh * 512 : (oh + 1) * 512],
                                a2[:, hc, tcc * P : (tcc + 1) * P],
                                woT[:, hc, oh * 512 : (oh + 1) * 512],
                                start=(hc == 0),
                                stop=(hc == OC - 1),
                            )
                    hid_t = fin.tile([P, H], F32, tag="hid_t")
                    nc.sync.dma_start(hid_t[:], hid[tcc * P : (tcc + 1) * P, :])
                    nc.vector.tensor_tensor(hid_t[:], hid_t[:], bo_rep[:], ADD)
                    x_t = fin.tile([P, H], F32, tag="x_t")
                    xsum = lnstat.tile([P, 1], F32, tag="xsum")
                    nc.vector.scalar_tensor_tensor(
                        x_t[:], ps[:], c_o[:], hid_t[:], MULT, ADD,
                        accum_out=xsum[:],
                    )
                    mu = lnstat.tile([P, 1], F32, tag="mu")
                    nc.vector.tensor_scalar(mu[:], xsum[:], 1.0 / H, None, MULT)
                    xc_t = fin.tile([P, H], F32, tag="xc_t")
                    varsum = lnstat.tile([P, 1], F32, tag="varsum")
                    nc.vector.tensor_scalar(xc_t[:], x_t[:], mu[:], None, SUB)
                    # square into the spent hid_t slot; only accum_out matters
                    nc.vector.scalar_tensor_tensor(
                        hid_t[:], xc_t[:], 1.0, xc_t[:], MULT, MULT,
                        accum_out=varsum[:],
                    )
                    std = lnstat.tile([P, 1], F32, tag="std")
                    nc.scalar.activation(
                        std[:], varsum[:], Sqrt, scale=1.0 / H, bias=eps_t[:]
                    )
                    rstd = lnstat.tile([P, 1], F32, tag="rstd")
                    nc.vector.reciprocal(rstd[:], std[:])
                    nc.vector.tensor_scalar(xc_t[:], xc_t[:], rstd[:], None, MULT)
                    nc.vector.tensor_tensor(xc_t[:], xc_t[:], g_rep[:], MULT)
                    nc.vector.tensor_tensor(xc_t[:], xc_t[:], beta_rep[:], ADD)
                    nc.sync.dma_start(out_attn[tcc * P : (tcc + 1) * P, :], xc_t[:])

    nc.compile()
    return nc


_CACHE = {}


def _get_nc(mask_nonzero):
    key = bool(mask_nonzero)
    if key not in _CACHE:
        _CACHE[key] = build(key)
    return _CACHE[key]


def kernel(
    hidden_states,
    attention_mask,
    Wq,
    bq,
    Wk,
    bk,
    Wv,
    bv,
    Wo,
    bo,
    ln_gamma,
    ln_beta,
    trace=False,
    **trace_kwargs,
):
    hidden_states = np.asarray(hidden_states, dtype=np.float32)
    attention_mask = np.asarray(attention_mask, dtype=np.float32)
    B = hidden_states.shape[0]
    assert B == 8 and hidden_states.shape[1:] == (S, H)

    mask_nonzero = bool(np.any(attention_mask != 0.0))
    nc = _get_nc(mask_nonzero)

    shared = {
        "Wq": np.ascontiguousarray(Wq, np.float32),
        "Wk": np.ascontiguousarray(Wk, np.float32),
        "Wv": np.ascontiguousarray(Wv, np.float32),
        "Wo": np.ascontiguousarray(Wo, np.float32),
        "bq": np.ascontiguousarray(bq, np.float32),
        "bk": np.ascontiguousarray(bk, np.float32),
        "bv": np.ascontiguousarray(bv, np.float32),
        "bo": np.ascontiguousarray(bo, np.float32),
        "ln_gamma": np.ascontiguousarray(ln_gamma, np.float32),
        "ln_beta": np.ascontiguousarray(ln_beta, np.float32),
    }
    in_maps = []
    for b in range(B):
        m = dict(shared)
        m["hidden"] = np.ascontiguousarray(hidden_states[b])
        m["mask"] = np.ascontiguousarray(
            np.broadcast_to(attention_mask[b], (1, 1, S)).reshape(S)
        )
        in_maps.append(m)

    res = run_bass_kernel_spmd(
        nc, in_maps, core_ids=list(range(8)), trace=trace, **trace_kwargs
    )
    attn = np.stack([res.results[b]["attn_out"] for b in range(B)])
    scores = np.stack([res.results[b]["scores"] for b in range(B)])
    probs = np.stack([res.results[b]["probs"] for b in range(B)])
    kernel.last_results = res
    return attn, scores, probs


# revision 16
# speedup vs baseline: 2.9492x; 1.0190x over previous
"""BertAttention (quantized, eval) Trainium2 kernel.

Data-parallel over batch: 8 batch elements -> 8 NeuronCores, one full
attention layer per core, no collectives.  Returns the same tuple as the
reference: (attention_output, scores, probs).

Key ideas:
- Every fake-quantized tensor (weights, q, k, v, probs, ctx) is
  round(x/s) * s with round(x/s) an integer in [-127, 127].  We carry the
  integers (exact in bf16) through the TensorEngine, so QK^T, PV and the
  output projection are exact integer matmuls accumulated in f32 PSUM
  (max |sum| < 2^24); the f32 scales are applied in the epilogues.
- round-half-to-even == (x + 1.5*2^23) - 1.5*2^23 in f32, one 2-op
  tensor_scalar instruction (matches jnp.round for |x| <= 2^22).
- scores are computed in both [q,k] layout (scores/probs outputs, softmax
  row sums via the ACT engine's accum_out) and [k,q] layout (so the PV
  matmul gets probs^T without transposing the 64MB probs tensor).  Both
  layouts reuse the same Q^T / K^T integer operands.
"""

import os
import sys

sys.path.insert(0, "/opt/trn_rl_repo")
# 256B DRAM pages shatter DMA descriptors (2.5KB packets, ~225GB/s);
# 4KB pages let 4KB rows move as whole descriptors.
os.environ.setdefault("NEURON_SCRATCHPAD_PAGE_SIZE", "4096")

import numpy as np

import concourse.bass as bass
import concourse.bass_isa as bass_isa
import concourse.tile as tile
from concourse import bacc, mybir
from concourse.bass_utils import run_bass_kernel_spmd
from concourse.masks import make_identity

H = 1024
S = 1024
NH = 16
D = 64
P = 128
OC = H // P  # 8 chunks of 128 output channels
TC = S // P  # 8 chunks of 128 tokens
F32 = mybir.dt.float32
F32R = mybir.dt.float32r
BF16 = mybir.dt.bfloat16

CLIP = np.float32(2.5)
N_LEVELS = np.float32(127.0)
S_ACT = np.float32(CLIP / N_LEVELS)  # activation quant step
INV_S_ACT = float(np.float32(1.0) / S_ACT)  # ~50.8
C_QK = float(np.float32(S_ACT * S_ACT) / np.float32(8.0))  # sqrt(D) == 8
C_MAGIC = 12582912.0  # 1.5 * 2**23: f32 round-to-nearest-even bias
LN_EPS = 1e-12

ADD = mybir.AluOpType.add
SUB = mybir.AluOpType.subtract
MULT = mybir.AluOpType.mult
MIN = mybir.AluOpType.min
MAX = mybir.AluOpType.max
BYPASS = mybir.AluOpType.bypass
Exp = mybir.ActivationFunctionType.Exp
Copy = mybir.ActivationFunctionType.Copy
Identity = mybir.ActivationFunctionType.Identity
Sqrt = mybir.ActivationFunctionType.Sqrt
XY_AXIS = mybir.AxisListType.XY


def _transpose_to(nc, psum_pool, dst, src, ident):
    """PE-transpose [P, C, J*P] -> [P, J, C*P]: each 128x128 tile
    src[:, c, j*P:+P] lands transposed in dst[:, j, c*P:+P]."""
    n_c = src.shape[1]
    n_j = src.shape[2] // P
    for c in range(n_c):
        for j in range(n_j):
            pt = psum_pool.tile([P, P], F32, tag="tr_psum")
            nc.tensor.transpose(pt[:], src[:, c, j * P : (j + 1) * P], ident)
            nc.any.tensor_copy(out=dst[:, j, c * P : (c + 1) * P], in_=pt[:])


def build(mask_nonzero: bool):
    nc = bacc.Bacc("TRN2", target_bir_lowering=False, debug=False, num_devices=8)

    hid = nc.dram_tensor("hidden", [S, H], F32, kind="ExternalInput").ap()
    msk = nc.dram_tensor("mask", [S], F32, kind="ExternalInput").ap()
    w_dr = {}
    b_dr = {}
    for nm, w_name, b_name in (
        ("q", "Wq", "bq"),
        ("k", "Wk", "bk"),
        ("v", "Wv", "bv"),
        ("o", "Wo", "bo"),
    ):
        w_dr[nm] = nc.dram_tensor(w_name, [H, H], F32, kind="ExternalInput").ap()
        b_dr[nm] = nc.dram_tensor(b_name, [H], F32, kind="ExternalInput").ap()
    g_ln = nc.dram_tensor("ln_gamma", [H], F32, kind="ExternalInput").ap()
    beta_ln = nc.dram_tensor("ln_beta", [H], F32, kind="ExternalInput").ap()

    out_attn = nc.dram_tensor("attn_out", [S, H], F32, kind="ExternalOutput").ap()
    out_scores = nc.dram_tensor("scores", [NH, S, S], F32, kind="ExternalOutput").ap()
    out_probs = nc.dram_tensor("probs", [NH, S, S], F32, kind="ExternalOutput").ap()

    with tile.TileContext(nc) as tc:
        with tc.tile_pool(name="pers", bufs=1) as pers:
            # -------- persistent tensors (span several phases) --------
            qT = pers.tile([P, OC, S], BF16, tag="qT")  # q^T ints [o, t]
            kT = pers.tile([P, OC, S], BF16, tag="kT")
            v_int = pers.tile([P, TC, H], BF16, tag="v_int")  # v ints [t, o]
            a2 = pers.tile([P, OC, S], BF16, tag="a2")  # ctx_q^T ints [H, t]
            woT = pers.tile([P, OC, H], BF16, tag="woT")  # Wo'^T ints [i, o]
            ident = pers.tile([P, P], F32, tag="ident")
            make_identity(nc, ident[:])
            # per-o-channel bias carriers ([o%128, oc]) pre-scaled by 1/s
            b50 = {}
            for nm in ("q", "k"):
                braw = pers.tile([P, OC], F32, tag=f"braw_{nm}")
                nc.sync.dma_start(braw[:], b_dr[nm].rearrange("(oc p) -> p oc", p=P))
                b50[nm] = pers.tile([P, OC], F32, tag=f"b50_{nm}", name=f"b50_{nm}")
                nc.vector.tensor_scalar(b50[nm][:], braw[:], INV_S_ACT, None, MULT)
            bv_row = pers.tile([1, H], F32, tag="bv_row")
            nc.sync.dma_start(bv_row[:], b_dr["v"][None, :])
            bv50 = pers.tile([1, H], F32, tag="bv50")
            nc.vector.tensor_scalar(bv50[:], bv_row[:], INV_S_ACT, None, MULT)
            bv50_rep = pers.tile([P, H], F32, tag="bv50_rep")
            nc.gpsimd.partition_broadcast(bv50_rep[:], bv50[0:1, :])
            bo_row = pers.tile([1, H], F32, tag="bo_row")
            nc.sync.dma_start(bo_row[:], b_dr["o"][None, :])
            g_row = pers.tile([1, H], F32, tag="g_row")
            nc.sync.dma_start(g_row[:], g_ln[None, :])
            beta_row = pers.tile([1, H], F32, tag="beta_row")
            nc.sync.dma_start(beta_row[:], beta_ln[None, :])
            if mask_nonzero:
                mask_pp = pers.tile([P, TC], F32, tag="mask_pp")  # [k%128, kc]
                nc.sync.dma_start(mask_pp[:], msk.rearrange("(kc p) -> p kc", p=P))
                mask_row = pers.tile([1, S], F32, tag="mask_row")
                nc.sync.dma_start(mask_row[:], msk[None, :])
                maskc = pers.tile([1, S], F32, tag="maskc")  # mask / C_QK
                nc.vector.tensor_scalar(maskc[:], mask_row[:], 1.0 / C_QK, None, MULT)
                ones_row = pers.tile([1, S], F32, tag="ones_row")
                nc.vector.memset(ones_row[:], 1.0)
            # weight scales (s_w = absmax/127), one [P,1] carrier per weight
            w_scale = {
                nm: pers.tile([P, 1], F32, tag=f"ws_{nm}", name=f"ws_{nm}")
                for nm in w_dr
            }

            # -------- phase A/B: x^T, quantized weights, projections --
            with (
                tc.tile_pool(name="xT_pool", bufs=1) as xTp,
                tc.tile_pool(name="tr_psum", bufs=4, space="PSUM") as trp,
                tc.tile_pool(name="mm_psum", bufs=4, space="PSUM") as mmp,
                tc.tile_pool(name="epi", bufs=2) as epi,
                tc.tile_pool(name="wstage", bufs=1) as wsp,
                tc.tile_pool(name="wT_pool", bufs=1) as wTp,
                tc.tile_pool(name="stats", bufs=1) as stats,
            ):
                xT = xTp.tile([P, OC, S], F32R, tag="xT")  # x^T [i, t]
                x_sb = wsp.tile([P, TC, H], F32, tag="w_sb", name="x_sb")
                nc.sync.dma_start(x_sb[:], hid.rearrange("(t p) i -> p t i", p=P))
                _transpose_to(nc, trp, xT, x_sb, ident[:])

                for nm in ("o", "v", "q", "k"):
                    # ---- load + per-tensor quantize (in place) ----
                    w_sb = wsp.tile([P, OC, H], F32, tag="w_sb")
                    nc.sync.dma_start(
                        w_sb[:], w_dr[nm].rearrange("(oc p) i -> p oc i", p=P)
                    )
                    amax_p = stats.tile([P, 1], F32, tag=f"amax_p_{nm}")
                    nc.vector.tensor_reduce(
                        amax_p[:], w_sb[:], XY_AXIS, MAX, apply_absolute_value=True
                    )
                    amax = stats.tile([P, 1], F32, tag=f"amax_{nm}")
                    nc.gpsimd.partition_all_reduce(
                        amax[:], amax_p[:], channels=P, reduce_op=bass_isa.ReduceOp.max
                    )
                    nc.vector.tensor_scalar(
                        w_scale[nm][:], amax[:], 1.0 / 127.0, None, MULT
                    )
                    inv_s = stats.tile([P, 1], F32, tag=f"invs_{nm}")
                    nc.vector.reciprocal(inv_s[:], amax[:])
                    nc.vector.tensor_scalar(inv_s[:], inv_s[:], 127.0, None, MULT)
                    nc.vector.tensor_scalar(
                        w_sb[:], w_sb[:], inv_s[:], C_MAGIC, MULT, ADD
                    )
                    nc.scalar.activation(w_sb[:], w_sb[:], Copy, bias=-C_MAGIC)
                    # ---- transpose W' ints -> [i, o] ----
                    wT = wTp.tile([P, OC, H], F32R, tag="wT")
                    _transpose_to(nc, trp, wT, w_sb, ident[:])
                    if nm == "o":
                        nc.any.tensor_copy(out=woT[:], in_=wT[:])
                        continue
                    # swi = s_w / s_act  (fold weight scale + act quant step)
                    swi = stats.tile([P, 1], F32, tag=f"swi_{nm}")
                    nc.vector.tensor_scalar(
                        swi[:], w_scale[nm][:], INV_S_ACT, None, MULT
                    )
                    if nm in ("q", "k"):
                        out_int = qT if nm == "q" else kT
                        # GEMM -> [o_p, t_f]
                        for oc in range(OC):
                            for th in range(2):
                                ps = mmp.tile([P, 512], F32, tag="proj_ps")
                                for ic in range(OC):
                                    nc.tensor.matmul(
                                        ps[:],
                                        wT[:, ic, oc * P : (oc + 1) * P],
                                        xT[:, ic, th * 512 : (th + 1) * 512],
                                        start=(ic == 0),
                                        stop=(ic == OC - 1),
                                    )
                                t1 = epi.tile([P, 512], F32, tag="prj_t1")
                                nc.scalar.activation(
                                    t1[:],
                                    ps[:],
                                    Identity,
                                    scale=swi[:],
                                    bias=b50[nm][:, oc : oc + 1],
                                )
                                nc.vector.tensor_scalar(
                                    t1[:], t1[:], 127.0, -127.0, MIN, MAX
                                )
                                nc.vector.tensor_scalar(
                                    out_int[:, oc, th * 512 : (th + 1) * 512],
                                    t1[:],
                                    C_MAGIC,
                                    C_MAGIC,
                                    ADD,
                                    SUB,
                                )
                    else:  # v: GEMM -> [t_p, o_f]
                        for tcc in range(TC):
                            for oh in range(2):
                                ps = mmp.tile([P, 512], F32, tag="proj_ps")
                                for ic in range(OC):
                                    nc.tensor.matmul(
                                        ps[:],
                                        xT[:, ic, tcc * P : (tcc + 1) * P],
                                        wT[:, ic, oh * 512 : (oh + 1) * 512],
                                        start=(ic == 0),
                                        stop=(ic == OC - 1),
                                    )
                                t1 = epi.tile([P, 512], F32, tag="prj_t1")
                                nc.vector.scalar_tensor_tensor(
                                    t1[:],
                                    ps[:],
                                    swi[:],
                                    bv50_rep[:, oh * 512 : (oh + 1) * 512],
                                    MULT,
                                    ADD,
                                )
                                nc.vector.tensor_scalar(
                                    t1[:], t1[:], 127.0, -127.0, MIN, MAX
                                )
                                nc.vector.tensor_scalar(
                                    v_int[:, tcc, oh * 512 : (oh + 1) * 512],
                                    t1[:],
                                    C_MAGIC,
                                    C_MAGIC,
                                    ADD,
                                    SUB,
                                )

            # -------- per-head attention ------------------------------
            with (
                tc.tile_pool(name="rr_dram", bufs=2, space="DRAM") as rrdp,
                tc.tile_pool(name="l1_psum", bufs=2, space="PSUM") as p1pool,
                tc.tile_pool(name="l2_psum", bufs=1, space="PSUM") as p2pool,
                tc.tile_pool(name="ctx_psum", bufs=1, space="PSUM") as pcpool,
                tc.tile_pool(name="l1", bufs=3) as l1pool,
                tc.tile_pool(name="l2", bufs=3) as l2pool,
                tc.tile_pool(name="hstat", bufs=3) as hstat,
                tc.tile_pool(name="ctxepi", bufs=2) as ctxepi,
            ):
                ctx_tiles = {}
                head_state = {}

                def l1_gen(h):
                    hp = h % 2
                    oc_h = h // 2
                    plo = D * hp
                    qT_h = qT[plo : plo + D, oc_h, :]  # [64, 1024] ints
                    kT_h = kT[plo : plo + D, oc_h, :]

                    rs = hstat.tile([P, TC], F32, tag="rs", name="rs")
                    rr = hstat.tile([P, TC], F32, tag="rr", name="rr")
                    rr50 = hstat.tile([P, TC], F32, tag="rr50", name="rr50")
                    rrT = hstat.tile([1, S], F32, tag="rrT", name="rrT")

                    # ---- layout 1: scores [q, k] ----
                    for qc in range(TC):
                        yield
                        ps1 = p1pool.tile([P, S], F32, tag="ps1", name="ps1")
                        for kh in range(2):
                            nc.tensor.matmul(
                                ps1[:, kh * 512 : (kh + 1) * 512],
                                qT_h[:, qc * P : (qc + 1) * P],
                                kT_h[:, kh * 512 : (kh + 1) * 512],
                                start=True,
                                stop=not mask_nonzero,
                            )
                            if mask_nonzero:
                                nc.tensor.matmul(
                                    ps1[:, kh * 512 : (kh + 1) * 512],
                                    ones_row[0:1, qc * P : (qc + 1) * P],
                                    maskc[0:1, kh * 512 : (kh + 1) * 512],
                                    start=False,
                                    stop=True,
                                    skip_group_check=True,
                                )
                        sc_t = l1pool.tile([P, S], F32, tag="sc")
                        nc.scalar.activation(sc_t[:], ps1[:], Copy, scale=C_QK)
                        nc.sync.dma_start(
                            out_scores[h, qc * P : (qc + 1) * P, :], sc_t[:]
                        )
                        un_t = l1pool.tile([P, S], F32, tag="un")
                        nc.scalar.activation(
                            un_t[:],
                            ps1[:],
                            Exp,
                            scale=C_QK,
                            accum_out=rs[:, qc : qc + 1],
                        )
                        nc.vector.reciprocal(rr[:, qc : qc + 1], rs[:, qc : qc + 1])
                        pr_t = l1pool.tile([P, S], F32, tag="pr")
                        nc.vector.tensor_scalar(
                            pr_t[:], un_t[:], rr[:, qc : qc + 1], None, MULT
                        )
                        nc.sync.dma_start(
                            out_probs[h, qc * P : (qc + 1) * P, :], pr_t[:]
                        )

                    nc.vector.tensor_scalar(rr50[:], rr[:], INV_S_ACT, None, MULT)
                    # scatter [128, TC] -> flat [S] (t = qc*128 + p) via DRAM
                    rr_d = rrdp.tile([S], F32, tag="rr_d")
                    with nc.allow_non_contiguous_dma(reason="tiny per-head stats"):
                        nc.sync.dma_start(
                            rr_d[:].rearrange("(c p) -> p c", p=P), rr50[:, :]
                        )
                    nc.sync.dma_start(rrT[:], rr_d[None, :])
                    rrT_rep = hstat.tile([P, S], F32, tag="rrT_rep", name="rrT_rep")
                    nc.gpsimd.partition_broadcast(rrT_rep[:], rrT[0:1, :])
                    head_state[h] = rrT_rep

                def l2_gen(h):
                    hp = h % 2
                    oc_h = h // 2
                    plo = D * hp
                    qT_h = qT[plo : plo + D, oc_h, :]
                    kT_h = kT[plo : plo + D, oc_h, :]
                    rrT_rep = head_state.pop(h)

                    # ---- layout 2: scores^T [k, q] feeding PV ----
                    if hp == 0:
                        ctx_tiles[oc_h] = pcpool.tile(
                            [P, S], F32, tag="ctx_ps", name="ctx_ps"
                        )
                    ctx_ps = ctx_tiles[oc_h]
                    for kc in range(TC):
                        yield
                        ps2 = p2pool.tile([P, S], F32, tag="ps2", name="ps2")
                        for qh in range(2):
                            nc.tensor.matmul(
                                ps2[:, qh * 512 : (qh + 1) * 512],
                                kT_h[:, kc * P : (kc + 1) * P],
                                qT_h[:, qh * 512 : (qh + 1) * 512],
                                start=True,
                                stop=True,
                            )
                        un2_t = l2pool.tile([P, S], F32, tag="un2", name="un2")
                        if mask_nonzero:
                            nc.scalar.activation(
                                un2_t[:],
                                ps2[:],
                                Exp,
                                scale=C_QK,
                                bias=mask_pp[:, kc : kc + 1],
                            )
                        else:
                            nc.scalar.activation(un2_t[:], ps2[:], Exp, scale=C_QK)
                        t2_t = l2pool.tile([P, S], F32, tag="t2", name="t2")
                        aT_t = l2pool.tile([P, S], BF16, tag="aT", name="aT")
                        for qh in range(2):
                            sl = slice(qh * 512, (qh + 1) * 512)
                            eng = nc.gpsimd if qh == 0 else nc.vector
                            eng.tensor_tensor(
                                t2_t[:, sl], un2_t[:, sl], rrT_rep[:, sl], MULT
                            )
                            nc.vector.tensor_scalar(
                                aT_t[:, sl], t2_t[:, sl], C_MAGIC, C_MAGIC, ADD, SUB
                            )
                            nc.tensor.matmul(
                                ctx_ps[plo : plo + D, sl],
                                v_int[:, kc, h * D : (h + 1) * D],
                                aT_t[:, sl],
                                start=(kc == 0),
                                stop=(kc == TC - 1),
                                skip_group_check=True,
                            )

                    if hp == 1:
                        # fake-quant ctx^T pair -> a2[:, oc_h, :] (ints, bf16)
                        ctx_tiles.pop(oc_h)
                        c_pv = float(np.float32(S_ACT * S_ACT) * np.float32(INV_S_ACT))
                        e1 = ctxepi.tile([P, S], F32, tag="ce1", name="e1")
                        nc.vector.tensor_scalar(
                            e1[:], ctx_ps[:], c_pv, 127.0, MULT, MIN
                        )
                        nc.vector.tensor_scalar(
                            e1[:], e1[:], -127.0, C_MAGIC, MAX, ADD
                        )
                        nc.vector.tensor_scalar(
                            a2[:, oc_h, :], e1[:], C_MAGIC, None, SUB
                        )

                # software pipeline: interleave L1(h) and L2(h-1) at tile
                # granularity so no engine's in-order stream stalls long
                def drive(g1, g2):
                    while True:
                        done1 = done2 = False
                        try:
                            next(g1)
                        except StopIteration:
                            done1 = True
                        try:
                            next(g2)
                        except StopIteration:
                            done2 = True
                        if done1 and done2:
                            return

                def empty_gen():
                    return iter(())

                prev = empty_gen()
                for h in range(NH):
                    drive(l1_gen(h), prev)
                    prev = l2_gen(h)
                drive(empty_gen(), prev)

            # -------- output projection + residual + LayerNorm --------
            with (
                tc.tile_pool(name="o_psum", bufs=2, space="PSUM") as opool,
                tc.tile_pool(name="fin", bufs=3) as fin,
                tc.tile_pool(name="rows", bufs=1) as rows,
                tc.tile_pool(name="lnstat", bufs=2) as lnstat,
            ):
                bo_rep = rows.tile([P, H], F32, tag="bo_rep")
                nc.gpsimd.partition_broadcast(bo_rep[:], bo_row[0:1, :])
                g_rep = rows.tile([P, H], F32, tag="g_rep")
                nc.gpsimd.partition_broadcast(g_rep[:], g_row[0:1, :])
                beta_rep = rows.tile([P, H], F32, tag="beta_rep")
                nc.gpsimd.partition_broadcast(beta_rep[:], beta_row[0:1, :])
                c_o = lnstat.tile([P, 1], F32, tag="c_o")  # s_ctx * s_wo
                nc.vector.tensor_scalar(
                    c_o[:], w_scale["o"][:], float(S_ACT), None, MULT
                )
                eps_t = lnstat.tile([P, 1], F32, tag="eps_t")
                nc.vector.memset(eps_t[:], LN_EPS)
                for tcc in range(TC):
                    ps = opool.tile([P, H], F32, tag="ops")
                    for oh in range(2):
                        for hc in range(OC):
                            nc.tensor.matmul(
                                ps[:, oh * 512 : (oh + 1) * 512],
                                a2[:, hc, tcc * P : (tcc + 1) * P],
                                woT[:, hc, oh * 512 : (oh + 1) * 512],
                                start=(hc == 0),
                                stop=(hc == OC - 1),
                            )
                    hid_t = fin.tile([P, H], F32, tag="hid_t")
                    nc.sync.dma_start(hid_t[:], hid[tcc * P : (tcc + 1) * P, :])
                    nc.vector.tensor_tensor(hid_t[:], hid_t[:], bo_rep[:], ADD)
                    x_t = fin.tile([P, H], F32, tag="x_t")
                    xsum = lnstat.tile([P, 1], F32, tag="xsum")
                    nc.vector.scalar_tensor_tensor(
                        x_t[:], ps[:], c_o[:], hid_t[:], MULT, ADD,
                        accum_out=xsum[:],
                    )
                    mu = lnstat.tile([P, 1], F32, tag="mu")
                    nc.vector.tensor_scalar(mu[:], xsum[:], 1.0 / H, None, MULT)
                    xc_t = fin.tile([P, H], F32, tag="xc_t")
                    varsum = lnstat.tile([P, 1], F32, tag="varsum")
                    nc.vector.tensor_scalar(xc_t[:], x_t[:], mu[:], None, SUB)
                    # square into the spent hid_t slot; only accum_out matters
                    nc.vector.scalar_tensor_tensor(
                        hid_t[:], xc_t[:], 1.0, xc_t[:], MULT, MULT,
                        accum_out=varsum[:],
                    )
                    std = lnstat.tile([P, 1], F32, tag="std")
                    nc.scalar.activation(
                        std[:], varsum[:], Sqrt, scale=1.0 / H, bias=eps_t[:]
                    )
                    rstd = lnstat.tile([P, 1], F32, tag="rstd")
                    nc.vector.reciprocal(rstd[:], std[:])
                    nc.vector.tensor_scalar(xc_t[:], xc_t[:], rstd[:], None, MULT)
                    nc.vector.tensor_tensor(xc_t[:], xc_t[:], g_rep[:], MULT)
                    nc.vector.tensor_tensor(xc_t[:], xc_t[:], beta_rep[:], ADD)
                    nc.sync.dma_start(out_attn[tcc * P : (tcc + 1) * P, :], xc_t[:])

    nc.compile()
    return nc


_CACHE = {}


def _get_nc(mask_nonzero):
    key = bool(mask_nonzero)
    if key not in _CACHE:
        _CACHE[key] = build(key)
    return _CACHE[key]


def kernel(
    hidden_states,
    attention_mask,
    Wq,
    bq,
    Wk,
    bk,
    Wv,
    bv,
    Wo,
    bo,
    ln_gamma,
    ln_beta,
    trace=False,
    **trace_kwargs,
):
    hidden_states = np.asarray(hidden_states, dtype=np.float32)
    attention_mask = np.asarray(attention_mask, dtype=np.float32)
    B = hidden_states.shape[0]
    assert B == 8 and hidden_states.shape[1:] == (S, H)

    mask_nonzero = bool(np.any(attention_mask != 0.0))
    nc = _get_nc(mask_nonzero)

    shared = {
        "Wq": np.ascontiguousarray(Wq, np.float32),
        "Wk": np.ascontiguousarray(Wk, np.float32),
        "Wv": np.ascontiguousarray(Wv, np.float32),
        "Wo": np.ascontiguousarray(Wo, np.float32),
        "bq": np.ascontiguousarray(bq, np.float32),
        "bk": np.ascontiguousarray(bk, np.float32),
        "bv": np.ascontiguousarray(bv, np.float32),
        "bo": np.ascontiguousarray(bo, np.float32),
        "ln_gamma": np.ascontiguousarray(ln_gamma, np.float32),
        "ln_beta": np.ascontiguousarray(ln_beta, np.float32),
    }
    in_maps = []
    for b in range(B):
        m = dict(shared)
        m["hidden"] = np.ascontiguousarray(hidden_states[b])
        m["mask"] = np.ascontiguousarray(
            np.broadcast_to(attention_mask[b], (1, 1, S)).reshape(S)
        )
        in_maps.append(m)

    res = run_bass_kernel_spmd(
        nc, in_maps, core_ids=list(range(8)), trace=trace, **trace_kwargs
    )
    attn = np.stack([res.results[b]["attn_out"] for b in range(B)])
    scores = np.stack([res.results[b]["scores"] for b in range(B)])
    probs = np.stack([res.results[b]["probs"] for b in range(B)])
    kernel.last_results = res
    return attn, scores, probs
